# revision 1
# baseline (speedup 1.0000x reference)
"""Trainium2 Bass kernel for a CapsuleNet dynamic-routing layer.

Math (per batch element b):
    u_hat[b,i,o,d] = sum_k W[i,o,d,k] * x[b,i,k]      # B=256, IC=1152, OC=10, OD=16, ID=8
    b_log = 0
    for it in 0..2:
        c = softmax(b_log, axis=o)
        s[b,o,d] = sum_i c[b,i,o] * u_hat[b,i,o,d]
        v = squash(s)
        if it < 2: b_log += sum_d u_hat * v

Sharding: data-parallel over B across 8 cores (32 local rows), W replicated.

Per-core layout: partition axis p = bp*16 + ii (bp = b%8, ii = i%16); the
u_hat build contracts q = ii*8 + k with a host-prepacked block-diagonal x
(lhsT) against the W stack (rhs), one matmul per (iblk, b-pair), built in two
b-pair halves so iter-0's agreement for b-blocks 0/1 overlaps the second
half.  u_hat lives in SBUF as [128(p), 72(iblk), 4(bblk), 160(o*16+d)] fp16.

Routing iterations keep c additively split as c = 1/OC + delta: the s0 term
0.1*sum_i u_hat comes from a single persistent pass-1 psum (computed once
from xt/ws directly), and only the delta-weighted sum runs per iteration.
delta is computed in f32 (no cancellation), scaled by DSCALE and stored fp8.
It reaches the block-diagonal stage-1 lhsT layout via a DRAM bounce: one
store through a 4D affine access pattern (DRAM is linear, so the
partition-group-dependent diagonal offset is just a stride) and one linear
load back - 2 SWDGE DMAs instead of 8 partition-group scatters.  Stage-1 is
then fp8(lhsT) x fp16(rhs) matmuls accumulating 256*sum_i delta*u_hat per
b-block; a psum-preload matmul injects s0 and the o80 extraction constant
carries the 1/DSCALE.

The agreement step (sum_d u*v) is a fp16 DVE multiply (2x mode) +
in-place pairwise-add tree, split 60 i-blocks on DVE / 12 on GPSIMD, and
each step's agreement is emitted one pipeline step late so it executes
exactly while that step's stage-1 matmuls occupy the PE.  Iter-2 steps
(agreement-free) run after iter-1 with their softmaxes two steps behind
their producers.  PSUM drains of the u_hat build are spread DVE/ACT/GPSIMD
("daad"/"aada" patterns tuned against the TimelineSim cost model).
"""

import sys

sys.path.insert(0, "/opt/trn_rl_repo")

from contextlib import ExitStack

import numpy as np

import bass_rust
import concourse.bass as bass
import concourse.tile as tile
from concourse import mybir

# fp16 (not bf16): same PE/DVE throughput for 2-byte dtypes, 4 extra mantissa
# bits; all tensors here have magnitudes well inside fp16 normal range.
BF = mybir.dt.float16
F8 = mybir.dt.float8e4
F32 = mybir.dt.float32
AX = mybir.AxisListType
AF = mybir.ActivationFunctionType
# c is routed through the blockdiag as delta = c - 1/OC, scaled by DSCALE so
# fp8e4m3 holds it with ~0.8% relative error on a term that is itself only
# ~1e-3 of s; the 1/DSCALE is folded into the o80 extraction constant.
DSCALE = 1024.0

N_CORES = 8
B_FULL, IC, OC, OD, ID = 256, 1152, 10, 16, 8
B_LOC = B_FULL // N_CORES          # 32
NIB = IC // 16                     # 72 i-blocks of 16
NBB = B_LOC // 8                   # 4 b-blocks of 8
F = OC * OD                        # 160


def _squash(nc, smp, ps, scale, vout, P=8):
    """vout = squash(scale * ps) with ps a [P, 160] psum slab (f32).

    squash(s) = (n^2/(1+n^2)) * s/(n + 1e-8),  n = ||s||_2 over d.
    """
    sq = smp.tile([P, F], F32, tag=f"sq{P}")
    nc.scalar.activation(sq[:], ps[:], AF.Square, scale=float(scale))
    n2 = smp.tile([P, OC], F32, tag=f"n2{P}")
    nc.vector.tensor_reduce(
        n2[:], sq[:].rearrange("p (o d) -> p o d", d=OD), axis=AX.X,
        op=mybir.AluOpType.add)
    n1 = smp.tile([P, OC], F32, tag=f"n1{P}")
    nc.scalar.add(n1[:], n2[:], 1.0)
    r1 = smp.tile([P, OC], F32, tag=f"r1{P}")
    nc.vector.reciprocal(r1[:], n1[:])
    sn = smp.tile([P, OC], F32, tag=f"sn{P}")
    nc.scalar.sqrt(sn[:], n2[:])
    sne = smp.tile([P, OC], F32, tag=f"sne{P}")
    nc.vector.tensor_scalar_add(sne[:], sn[:], 1e-8)
    r2 = smp.tile([P, OC], F32, tag=f"r2{P}")
    nc.vector.reciprocal(r2[:], sne[:])
    f1 = smp.tile([P, OC], F32, tag=f"f1{P}")
    nc.vector.tensor_mul(f1[:], n2[:], r1[:])
    nc.vector.tensor_mul(f1[:], f1[:], r2[:])
    if scale != 1.0:
        nc.scalar.mul(f1[:], f1[:], float(scale))
    nc.vector.tensor_mul(
        vout[:].rearrange("p (o d) -> p o d", d=OD),
        ps[:].rearrange("p (o d) -> p o d", d=OD),
        f1[:].unsqueeze(-1).broadcast_to((P, OC, OD)))


def _split_multiwait(nc):
    """The walrus in this container encodes at most ONE semaphore wait on
    Matmult/Ldweights and HWDGE DMACopy instructions ("Too many sync wait
    commands").  Hoist excess waits onto same-engine NoOps placed directly
    before the instruction - position-identical semantics, ~2 cycles each.
    SWDGE (Pool-queue) DMAs handle multi-waits fine and are left alone.
    """
    for fn in nc.m.functions:
        for bb in fn.blocks:
            out = []
            k = 0
            for ins in bb.instructions:
                si = ins.sync_info
                waits = list(si.on_wait) if si is not None and si.on_wait else []
                limit = 1
                if ins.opcode == "DMACopy":
                    q = str(getattr(ins, "queue", "") or "")
                    if "HW" in q and len(waits) > 1:
                        # HWDGE queue instructions can't be gated by engine
                        # NoOps; the kernel must be structured to avoid this.
                        raise AssertionError(
                            f"HWDGE DMA {ins.name} has {len(waits)} waits: {ins}")
                if len(waits) > limit:
                    for w in waits[:-limit]:
                        nop = mybir.InstNoOp(name=f"{ins.name}-wn{k}", ins=[], outs=[])
                        k += 1
                        nop.engine = ins.engine
                        nop.sync_info = mybir.SyncInfo(on_wait=[w], on_update=[])
                        out.append(nop)
                    ins.sync_info = mybir.SyncInfo(
                        on_wait=waits[-limit:],
                        on_update=list(si.on_update) if si.on_update else [])
                out.append(ins)
            bb.instructions = out


def _strip_scatter_waits(nc, names):
    """The c-scatter DMAs ride the dedicated SP HWDGE queue (which cannot
    encode multiple waits).  Every wait the tile framework put on them is
    provably subsumed by their DVE-tick wait:

      - ACT wait (softmax exp wrote c_sb): the DVE softmax mult reads c_sb
        after exp, and DVE retires in order, so DVE>=tick(mult) implies the
        exp completed.
      - PE wait (cbt parity-buffer readers, i.e. stage-1 matmuls of bblk-2):
        the DVE mask-mult of bblk-2 waited on those matmuls and precedes the
        softmax mult of this bblk in DVE program order.
      - DMAHW sem-slot recycle (scatter 8-ago): stage-1 matmuls that the
        bblk-2 DVE mask-mult waited on themselves waited on those scatters.

    So: keep only the strongest DVE wait, drop the rest.  CoreSim's race
    detector runs on this stripped program and validates the claim.
    """
    names = set(names)
    for fn in nc.m.functions:
        for bb in fn.blocks:
            for ins in bb.instructions:
                if ins.name not in names:
                    continue
                si = ins.sync_info
                waits = list(si.on_wait) if si is not None and si.on_wait else []
                # Scatters 2..8 of each group carry no data wait at all (the
                # HWDGE queue is FIFO, so they order behind the first).
                dve = [w for w in waits if w.ant_name.startswith("DVE")]
                keep = [max(dve, key=lambda w: w.wait_value)] if dve else []
                ins.sync_info = mybir.SyncInfo(
                    on_wait=keep,
                    on_update=list(si.on_update) if si.on_update else [])


def build_program(split_waits=True):
    """split_waits=True applies the walrus 1-wait workaround (required for
    hardware compiles); CoreSim/TimelineSim need the unsplit program."""
    nc = bass.Bass()
    scatter_names = []
    bd_d = nc.declare_dram_parameter("bd", [2, 8, 128, 9, 2, 128], BF, isOutput=False)
    xt_d = nc.declare_dram_parameter("xt", [128, NIB, B_LOC], BF, isOutput=False)
    ws_d = nc.declare_dram_parameter("ws", [8, 128, 9, F], BF, isOutput=False)
    msk_d = nc.declare_dram_parameter("msk", [80, F], BF, isOutput=False)
    o80_d = nc.declare_dram_parameter("o80", [80, 8], BF, isOutput=False)
    sel_d = nc.declare_dram_parameter("sel", [8, 128], BF, isOutput=False)
    selb_d = nc.declare_dram_parameter("selb", [B_LOC, NBB, 128], BF, isOutput=False)
    out_d = nc.declare_dram_parameter("out", [B_LOC, F], F32, isOutput=True)
    # DRAM bounce scratches for the c-blockdiag build (one per parity).  The
    # store DMA writes c through a 4D affine AP that lands each partition
    # group's rows in its diagonal column block; DRAM is linear so the
    # partition-group-dependent offset is just another stride.
    e8_d = nc.declare_dram_parameter("e8", [B_LOC, NBB, 8], BF, isOutput=False)
    zcb_d = [nc.declare_dram_parameter(f"zcb{i}", [128 * 80 * NIB], F8,
                                       isOutput=True) for i in range(2)]

    def _zcb_diag(i):
        ap = zcb_d[i][:].copy()
        ap.ap = bass_rust.VecI64Pair(
            [[16 * 80 * NIB + 10 * NIB, 8], [80 * NIB, 16], [NIB, 10],
             [1, NIB]])
        return ap

    with ExitStack() as ctx:
        tc = ctx.enter_context(tile.TileContext(nc))
        st = ctx.enter_context(tc.tile_pool(name="st", bufs=1))
        bdp = ctx.enter_context(tc.tile_pool(name="bdp", bufs=2))
        y2p = ctx.enter_context(tc.tile_pool(name="y2p", bufs=2))
        y2q = ctx.enter_context(tc.tile_pool(name="y2q", bufs=1))
        tsp = ctx.enter_context(tc.tile_pool(name="tsp", bufs=2))
        tsq = ctx.enter_context(tc.tile_pool(name="tsq", bufs=1))
        mkp = ctx.enter_context(tc.tile_pool(name="mkp", bufs=2))
        vxp = ctx.enter_context(tc.tile_pool(name="vxp", bufs=2))
        smp = ctx.enter_context(tc.tile_pool(name="smp", bufs=4))
        pbig = ctx.enter_context(tc.tile_pool(name="pbig", bufs=4, space="PSUM"))
        psml = ctx.enter_context(tc.tile_pool(name="psml", bufs=3, space="PSUM"))
        ps0p = ctx.enter_context(tc.tile_pool(name="ps0p", bufs=1, space="PSUM"))

        # --- persistent tiles ---
        u_hat = st.tile([128, NIB, NBB, F], BF, tag="u_hat")
        blg = st.tile([128, NBB, OC, NIB], BF, tag="blg")
        c_sb = st.tile([128, NBB, OC, NIB], F8, tag="c_sb")
        cb0 = st.tile([128, 80, NIB], F8, tag="cb0")
        cb1 = st.tile([128, 80, NIB], F8, tag="cb1")
        msk_sb = st.tile([80, F], BF, tag="msk_sb")
        o80_sb = st.tile([80, 8], BF, tag="o80_sb")
        sel_sb = st.tile([8, 128], BF, tag="sel_sb")
        selb_sb = st.tile([B_LOC, NBB, 128], BF, tag="selb_sb")
        e8_sb = st.tile([B_LOC, NBB, 8], BF, tag="e8_sb")
        s0_sb = st.tile([B_LOC, F], BF, tag="s0_sb")
        xt_sb = st.tile([128, NIB, B_LOC], BF, tag="xt_sb")
        # ws kept resident for both pass 1 and pass 2; one tile per e-chunk
        # so consumers only wait on their own chunk's DMA.
        ws_sb = [st.tile([128, 9, F], BF, tag=f"ws_sb{e}", name=f"ws_sb{e}")
                 for e in range(8)]
        v32 = st.tile([B_LOC, F], BF, tag="v32")
        v8 = [st.tile([8, F], BF, tag=f"v8_{i}", name=f"v8_{i}") for i in range(NBB)]
        of8 = [st.tile([8, F], F32, tag=f"of8_{i}", name=f"of8_{i}") for i in range(NBB)]

        # --- input loads + zero-init of the c-blockdiag ---
        # xt + ws first so pass-1 can start immediately; ws alternates
        # between the two HWDGE queues to halve the load latency.
        nc.scalar.dma_start(out=xt_sb[:], in_=xt_d[:])
        for e in range(8):
            eng = nc.sync if e % 2 == 0 else nc.scalar
            eng.dma_start(out=ws_sb[e][:], in_=ws_d[e])
        nc.scalar.dma_start(out=msk_sb[:], in_=msk_d[:])
        nc.scalar.dma_start(out=o80_sb[:], in_=o80_d[:])
        nc.scalar.dma_start(out=sel_sb[:], in_=sel_d[:])
        nc.scalar.dma_start(out=selb_sb[:], in_=selb_d[:])
        nc.scalar.dma_start(out=e8_sb[:], in_=e8_d[:])
        nc.scalar.memzero(cb0[:])
        nc.scalar.memzero(cb1[:])

        # --- pass 1: iter-0 uniform-c reduction s0 = 0.1*sum_i u_hat computed
        # directly as x @ W over the full (i,k) contraction from xt/ws -- 72
        # matmuls (all 32 b at once) into one [32,160] psum.  Runs before the
        # build so iter-0's squash/agreement overlap the u_hat build below. ---
        ps0 = ps0p.tile([B_LOC, F], F32, tag="ps0")
        for e in range(8):
            for j in range(9):
                iblk = e * 9 + j
                nc.tensor.matmul(
                    ps0[:], lhsT=xt_sb[:, iblk, :], rhs=ws_sb[e][:, j, :],
                    start=(iblk == 0), stop=(iblk == NIB - 1))

        # --- pass 2: build u_hat, one matmul per (iblk, b-pair), K=128=(ii,k),
        # N=160.  Built in two b-pair halves (h) so the iter-0 agreement for
        # b-blocks 0/1 overlaps the h=1 build.  PSUM drains: DVE is idle
        # during h=0 (no agreement yet) and takes most of those; during h=1
        # DVE runs agreement, so ACT/GPSIMD take more. ---
        def build_half(h):
            drain_pat = {0: "daad", 1: "aada"}[h]
            for e in range(8):
                bdt = bdp.tile([128, 9, 2, 128], BF, tag="bdt")
                nc.gpsimd.dma_start(out=bdt[:], in_=bd_d[h, e])
                for j in range(9):
                    iblk = e * 9 + j
                    ps = pbig.tile([128, 2, F], F32, tag="pbig")
                    for t in range(2):
                        nc.tensor.matmul(
                            ps[:, t, :], lhsT=bdt[:, j, t, :],
                            rhs=ws_sb[e][:, j, :], start=True, stop=True)
                    eng = {"a": nc.scalar, "d": nc.vector, "p": nc.gpsimd}[
                        drain_pat[iblk % len(drain_pat)]]
                    if eng is nc.scalar:
                        eng.copy(u_hat[:, iblk, h * 2:(h + 1) * 2, :], ps[:])
                    else:
                        eng.tensor_copy(u_hat[:, iblk, h * 2:(h + 1) * 2, :], ps[:])

        # agreement chunking: DVE is ~3.8x faster per element than GPSIMD,
        # and GPSIMD also carries the c-scatter SWDGE overhead, so DVE takes
        # 64 i-blocks and GPSIMD 8.  The GPSIMD chunk can be deferred (in
        # iters 1/2) so it queues BEHIND the next b-block's c-scatters on the
        # Pool queue instead of delaying them.
        # iters 1/2 run with Pool busy (bounces), so DVE takes 60 of 72
        # slabs; during iter 0 Pool is mostly idle, so it takes 24.
        AGR_IT1 = ([slice(0, 20), slice(20, 40), slice(40, 60)], slice(60, 72))
        AGR_IT0 = AGR_IT1
        # the last agreement runs DVE-only: its GPSIMD chunk would sit on
        # the Pool queue exactly when the tail bounces need it
        AGR_LAST = AGR_IT1

        def agreement_bblk(bblk, first, sel_lhsT, v_rhs, defer_pool=False,
                           split=AGR_IT1):
            pvx = psml.tile([128, F], F32, tag="psml", name=f"pvx{bblk}")
            nc.tensor.matmul(
                pvx[:], lhsT=sel_lhsT, rhs=v_rhs, start=True, stop=True)
            vx = vxp.tile([128, F], BF, tag="vx", name=f"vx{bblk}")
            nc.scalar.copy(vx[:], pvx[:])

            def chunk(eng, sl):
                nj = sl.stop - sl.start
                # the GPSIMD chunk gets its own buffers so DVE's ring never
                # waits on a Pool sem through tile reuse
                if eng is nc.gpsimd:
                    y2t = y2q.tile([128, 18, F], BF, tag="y2q", name="y2tq")
                else:
                    y2t = y2p.tile([128, 24, F], BF, tag="y2", name="y2t")
                y2 = y2t[:, :nj, :]
                eng.tensor_mul(
                    y2, u_hat[:, sl, bblk, :],
                    vx[:].unsqueeze(1).broadcast_to((128, nj, F)))
                # in-place pairwise-add tree over d (fp16, 2x mode)
                y2v = y2.rearrange("p j (o d) -> p j o d", d=OD)
                eng.tensor_add(
                    y2v[:, :, :, 0:8], y2v[:, :, :, 0:8], y2v[:, :, :, 8:16])
                eng.tensor_add(
                    y2v[:, :, :, 0:4], y2v[:, :, :, 0:4], y2v[:, :, :, 4:8])
                eng.tensor_add(
                    y2v[:, :, :, 0:2], y2v[:, :, :, 0:2], y2v[:, :, :, 2:4])
                dst = blg[:, bblk, :, sl].transpose([0, 2, 1])
                if first:
                    eng.tensor_add(
                        dst, y2v[:, :, :, 0], y2v[:, :, :, 1])
                else:
                    if eng is nc.gpsimd:
                        tst = tsq.tile([128, 18, OC], BF, tag="tsq", name="tstq")
                    else:
                        tst = tsp.tile([128, 24, OC], BF, tag="ts", name="tst")
                    ts = tst[:, :nj, :]
                    eng.tensor_add(ts, y2v[:, :, :, 0], y2v[:, :, :, 1])
                    eng.tensor_add(dst, dst, ts)

            # pool chunk first: it then overlaps the same window as the DVE
            # chunks (the following step's stage-1) instead of queueing
            # behind the next bounce
            if split[1] is not None:
                chunk(nc.gpsimd, split[1])
            for sl in split[0]:
                chunk(nc.vector, sl)
            return None

        # --- iter 0: c uniform -> s = 0.1 * sum_i u_hat (accumulated above);
        # one 32-partition squash, then per-bblk agreement with vx picked out
        # of v32 by the per-bblk selector.  b-blocks 0/1 only need the h=0
        # half of u_hat, so they overlap the h=1 build. ---
        _squash(nc, smp, ps0[:], 0.1, v32, P=B_LOC)
        # s0 = 0.1 * sum_i u_hat, reused by iters 1/2 via psum preload
        nc.scalar.mul(s0_sb[:], ps0[:], 0.1)
        build_half(0)
        for bblk in (0, 1):
            agreement_bblk(bblk, first=True, split=AGR_IT0,
                           sel_lhsT=selb_sb[:, bblk, :], v_rhs=v32[:])
        build_half(1)
        # seed the DRAM bounce scratches with the zeroed blockdiag image;
        # emitted after the bd loads so they never delay the build, but well
        # before the first bounce needs them
        nc.gpsimd.dma_start(out=zcb_d[0][:].rearrange(
            "(p c i) -> p c i", p=128, c=80), in_=cb0[:])
        nc.gpsimd.dma_start(out=zcb_d[1][:].rearrange(
            "(p c i) -> p c i", p=128, c=80), in_=cb0[:])
        # b2/b3's iter-0 agreements are pushed into the iter-1/2 pipeline
        # queue below: they fill the DVE hole while the first iter-1 stage-1
        # matmuls run.
        agr_queue = [
            (lambda b=bblk: agreement_bblk(
                b, first=True, split=AGR_IT0,
                sel_lhsT=selb_sb[:, b, :], v_rhs=v32[:]))
            for bblk in (2, 3)]

        # --- iters 1, 2: fully per-bblk pipelines so PE stage-1 of one
        # b-block overlaps the DVE agreement/softmax of another ---
        # iter-2 steps carry no agreement, so interleaving them into iter-1's
        # stream fills DVE/Pool gaps: 2b0 only needs 1b0's blg final.
        # The whole agreement of step n is EMITTED inside step n+1 right
        # after its stage-1 matmuls: the DVE chunks then execute exactly
        # while DVE would otherwise idle waiting for step n+1's stage-1.
        def step_front(it, bblk):
            # softmax over o for this b-block (no max-sub: |logits| << 1),
            # in f32 so delta = c - 1/OC survives the cancellation
            c32 = smp.tile([128, OC, NIB], F32, tag="c32")
            nc.scalar.activation(
                c32[:], blg[:, bblk, :, :], AF.Exp)
            cf = c32[:].transpose([0, 2, 1])                # [p, i, o]
            sm = smp.tile([128, NIB], F32, tag="sm")
            nc.vector.tensor_reduce(
                sm[:], cf, axis=AX.X, op=mybir.AluOpType.add)
            rr = smp.tile([128, NIB], F32, tag="rr")
            nc.vector.reciprocal(rr[:], sm[:])
            nc.vector.tensor_mul(
                cf, cf, rr[:].unsqueeze(-1).broadcast_to((128, NIB, OC)))
            # delta8 = DSCALE*c - DSCALE/OC, written as fp8
            nc.scalar.activation(
                c_sb[:, bblk, :, :], c32[:], AF.Copy,
                bias=float(-DSCALE / OC), scale=float(DSCALE))

            # stage 1+2: s = diag(C^T U) via blockdiag-c, o-mask, reduce.
            # c reaches the blockdiag layout via a DRAM bounce: one store
            # with the diagonal 4D AP, one linear load back - 2 SWDGE
            # DMAs instead of 8 partition-group scatters.
            cbt = (cb0, cb1)[bblk % 2]
            nc.gpsimd.dma_start(
                out=_zcb_diag(bblk % 2), in_=c_sb[:, bblk, :, :])
            nc.gpsimd.dma_start(
                out=cbt[:], in_=zcb_d[bblk % 2][:].rearrange(
                    "(p c i) -> p c i", p=128, c=80))
            ps1 = pbig.tile([80, F], F32, tag="pbig", name=f"ps1_{bblk}")
            for j in range(NIB):
                nc.tensor.matmul(
                    ps1[:], lhsT=cbt[:, :, j],
                    rhs=u_hat[:, j, bblk, :],
                    start=(j == 0), stop=(j == NIB - 1))
            if agr_queue:
                agr_queue.pop(0)()
            return ps1

        def step_back(it, bblk, ps1):
            mskd = mkp.tile([80, F], BF, tag="mskd")
            nc.vector.tensor_mul(mskd[:], ps1[:], msk_sb[:])
            # psv = s0[bblk rows] (psum preload via e8 selector) +
            #       (1/DSCALE) * delta-term (folded into o80's values)
            psv = psml.tile([8, F], F32, tag="psml")
            nc.tensor.matmul(
                psv[:], lhsT=e8_sb[:, bblk, :], rhs=s0_sb[:],
                start=True, stop=False)
            nc.tensor.matmul(
                psv[:], lhsT=o80_sb[:], rhs=mskd[:], start=False, stop=True)
            if it == 1:
                _squash(nc, smp, psv, 1.0, v8[bblk])
                spl = AGR_LAST if bblk == 3 else AGR_IT1
                agr_queue.append(
                    lambda b=bblk, s=spl: agreement_bblk(
                        b, first=False, sel_lhsT=sel_sb[:],
                        v_rhs=v8[b][:], split=s))
            else:
                _squash(nc, smp, psv, 1.0, of8[bblk])
                out_pend.append(bblk)

        # Output DMAs are deferred past the loop: emitted inline, each
        # out-gen waits its squash on the in-order Pool queue and stalls the
        # NEXT step's bounce gens behind it.
        out_pend = []
        pend_back = None
        for it, bblk in ((1, 0), (1, 1), (1, 2), (1, 3),
                         (2, 0), (2, 1), (2, 2), (2, 3)):
            ps1 = step_front(it, bblk)
            if pend_back is not None:
                step_back(*pend_back)
                pend_back = None
            step_back(it, bblk, ps1)
        for b in out_pend:
            nc.gpsimd.dma_start(
                out=out_d[b * 8:(b + 1) * 8, :], in_=of8[b][:])

    if scatter_names:
        _strip_scatter_waits(nc, scatter_names)
    if split_waits:
        _split_multiwait(nc)
    return nc


def _host_inputs(x, W):
    """Per-core input maps from full x [256,1152,8] f32, W [1,1152,10,16,8] f32."""
    bf = np.float16
    W0 = np.asarray(W[0], dtype=np.float32)
    # ws[e, q=(ii,k), j, (o,d)] = W[(e*9+j)*16+ii, o, d, k]
    ws = np.ascontiguousarray(
        W0.reshape(8, 9, 16, OC, OD, ID).transpose(0, 2, 5, 1, 3, 4)
        .reshape(8, 128, 9, F)).astype(bf)
    msk = np.zeros((80, F), dtype=bf)
    for bpp in range(8):
        for o in range(OC):
            msk[bpp * 10 + o, o * OD:(o + 1) * OD] = 1.0
    o80 = np.zeros((80, 8), dtype=bf)
    for p in range(80):
        o80[p, p // 10] = 1.0 / DSCALE
    e8 = np.zeros((B_LOC, NBB, 8), dtype=bf)
    for b in range(NBB):
        for m in range(8):
            e8[b * 8 + m, b, m] = 1.0
    sel = np.zeros((8, 128), dtype=bf)
    for p in range(128):
        sel[p // 16, p] = 1.0
    # selb[b, bblk, p] = 1 iff b == bblk*8 + p//16  (vx_bblk = selb.T @ v32)
    selb = np.zeros((B_LOC, NBB, 128), dtype=bf)
    for bblk in range(NBB):
        for p in range(128):
            selb[bblk * 8 + p // 16, bblk, p] = 1.0

    in_maps = []
    for c in range(N_CORES):
        xc = np.asarray(x[c * B_LOC:(c + 1) * B_LOC], dtype=np.float32)
        # bd[e, q=(ii,k), j, bb, m=(bp,ii')] = x[bb*8+bp, (e*9+j)*16+ii, k] iff ii'==ii
        r = xc.reshape(NBB, 8, 8, 9, 16, ID)          # [bb, bp, e, j, ii, k]
        bd6 = np.zeros((8, 16, ID, 9, NBB, 8, 16), dtype=np.float32)
        for ii in range(16):
            # [bb, bp, e, j, k] -> [e, k, j, bb, bp]
            bd6[:, ii, :, :, :, :, ii] = r[:, :, :, :, ii, :].transpose(2, 4, 3, 0, 1)
        bd4 = bd6.reshape(8, 128, 9, NBB, 128)
        # [h, e, q, j, t, m]: b-pair halves h = (bb 0/1, bb 2/3) contiguous
        bd = np.ascontiguousarray(
            np.stack([bd4[:, :, :, 0:2, :], bd4[:, :, :, 2:4, :]], axis=0)
        ).astype(bf)
        # xt[q=(ii,k), iblk, b] = x[b, iblk*16+ii, k]
        xt = np.ascontiguousarray(
            xc.reshape(B_LOC, NIB, 16, ID).transpose(2, 3, 1, 0)
            .reshape(128, NIB, B_LOC)).astype(bf)
        in_maps.append(
            {"bd": bd, "xt": xt, "ws": ws, "msk": msk, "o80": o80,
             "sel": sel, "selb": selb, "e8": e8})
    return in_maps


_NC_CACHE = {}


def kernel(x, W):
    from concourse.bass_utils import run_bass_kernel_spmd

    if "nc" not in _NC_CACHE:
        _NC_CACHE["nc"] = build_program()
    nc = _NC_CACHE["nc"]
    in_maps = _host_inputs(x, W)
    res = run_bass_kernel_spmd(nc, in_maps, core_ids=list(range(N_CORES)))
    out = np.concatenate([r["out"] for r in res.results], axis=0)
    return out.reshape(B_FULL, OC, OD).astype(np.float32)


if __name__ == "__main__":
    nc = build_program()
    print("program built ok")



# revision 3
# speedup vs baseline: 1.1853x; 1.1853x over previous
"""Trainium2 Bass kernel for the CapsuleNet dynamic-routing layer, v2.

Math (per batch element b):
    u_hat[b,i,o,d] = sum_k W[i,o,d,k] * x[b,i,k]   # B=256, IC=1152, OC=10, OD=16, ID=8
    b_log = 0
    for it in 0..2:
        c = softmax(b_log, axis=o); s = sum_i c*u_hat; v = squash(s)
        if it < 2: b_log += sum_d u_hat * v

v2 design (vs the v1 DVE-agreement kernel):
  * Data-parallel over B across 8 cores (32 local rows), W replicated.
  * Partition layout p=(bp4, ii32): b-blocks of 4 (bb in 0..8), i-blocks of 32
    (j in 0..36).  All heavy contractions are fp8e4 DoubleRow matmuls on PE.
  * s is split s = s0 + delta-terms: s0 = 0.1*sum_i u_hat comes from an fp16
    pass-1 (x16 @ W16, full precision); everything delta-scaled (0.4% of s)
    runs in fp8 (error-tolerant).
  * The agreement (b_inc = sum_d u_hat*v) moves from DVE onto the PE via a
    transposed fp8 copy u_hatT[(o,d), (bp,ii)] built directly by ws8 x bd
    matmuls.  Agreement output lands as psum [128,(bp',o')=40+4] per (j,bb)
    whose 40 columns ARE the c-blockdiag positions for stage-1: no DRAM
    bounces at all.  4 extra M-columns compute sum_o b_inc/10 in the same
    matmul, so softmax is LINEARIZED (|logits|~4e-3): delta = (b_inc -
    mean_o)*0.1, computed by one DVE subtract + one mask-multiply.
  * Stage-1 s-increments accumulate across routing iterations in a [32,160]
    psum (delta_t are increments); one squash per round.
  * o in 10 = 8+2: u_hatT keeps (o<8,d) on 128 partitions (plane t0) and
    (o>=8,d) 4-up packed over j%4 (plane t1); the agreement DoubleRow matmul
    sums both planes with per-j variable-stride APs and per-class vd masks.
"""

import sys

sys.path.insert(0, "/opt/trn_rl_repo")

from contextlib import ExitStack

import numpy as np

import bass_rust
import concourse.bass as bass
import concourse.tile as tile
from concourse import mybir

BF = mybir.dt.float16
F8 = mybir.dt.float8e4
F32 = mybir.dt.float32
AX = mybir.AxisListType
AF = mybir.ActivationFunctionType
DR = mybir.MatmulPerfMode.DoubleRow

N_CORES = 8
B_FULL, IC, OC, OD, ID = 256, 1152, 10, 16, 8
B_LOC = B_FULL // N_CORES            # 32
F = OC * OD                          # 160
NJ = 36                              # i-blocks of 32
NJQ = 9                              # jq groups of 4 j-blocks
NBB = 8                              # b-blocks of 4
SW = 64.0                            # W scale into fp8
SV = 32.0                            # v scale into fp8
PSA = 2048.0                         # agreement psum scale = SW*SV
DSC = 1024.0                         # delta fp8 scale


def _squash(nc, smp, ps, scale, vout, P=B_LOC):
    """vout = squash(scale * ps) with ps a [P, 160] psum slab (f32).

    squash(s) = n2/((1+n2)(n+eps)) * s with n2 = |s|^2 per (b,o).
    """
    sq = smp.tile([P, F], F32, tag=f"sq{P}", name="sq")
    nc.scalar.activation(sq[:], ps[:], AF.Square, scale=float(scale))
    n2 = smp.tile([P, OC], F32, tag=f"n2{P}", name="n2")
    nc.vector.tensor_reduce(
        n2[:], sq[:].rearrange("p (o d) -> p o d", d=OD), axis=AX.X,
        op=mybir.AluOpType.add)
    n1 = smp.tile([P, OC], F32, tag=f"n1{P}", name="n1")
    nc.scalar.add(n1[:], n2[:], 1.0)
    sn = smp.tile([P, OC], F32, tag=f"sn{P}", name="sn")
    nc.scalar.sqrt(sn[:], n2[:])
    t1 = smp.tile([P, OC], F32, tag=f"t1{P}", name="t1")
    nc.vector.tensor_mul(t1[:], n1[:], sn[:])
    r1 = smp.tile([P, OC], F32, tag=f"r1{P}", name="r1")
    nc.vector.reciprocal(r1[:], t1[:])
    f1 = smp.tile([P, OC], F32, tag=f"f1{P}", name="f1")
    nc.vector.tensor_mul(f1[:], n2[:], r1[:])
    if scale != 1.0:
        nc.scalar.mul(f1[:], f1[:], float(scale))
    nc.vector.tensor_mul(
        vout[:].rearrange("p (o d) -> p o d", d=OD),
        ps[:].rearrange("p (o d) -> p o d", d=OD),
        f1[:].unsqueeze(-1).broadcast_to((P, OC, OD)))


def _split_multiwait(nc):
    """Walrus encodes at most ONE semaphore wait on Matmult/Ldweights and
    HWDGE DMACopy instructions.  Hoist excess waits onto same-engine NoOps."""
    for fn in nc.m.functions:
        for bb in fn.blocks:
            out = []
            k = 0
            for ins in bb.instructions:
                si = ins.sync_info
                waits = list(si.on_wait) if si is not None and si.on_wait else []
                limit = 1
                if ins.opcode == "DMACopy":
                    q = str(getattr(ins, "queue", "") or "")
                    if "HW" in q and len(waits) > 1:
                        raise AssertionError(
                            f"HWDGE DMA {ins.name} has {len(waits)} waits: {ins}")
                if len(waits) > limit:
                    for w in waits[:-limit]:
                        nop = mybir.InstNoOp(name=f"{ins.name}-wn{k}", ins=[], outs=[])
                        k += 1
                        nop.engine = ins.engine
                        nop.sync_info = mybir.SyncInfo(on_wait=[w], on_update=[])
                        out.append(nop)
                    ins.sync_info = mybir.SyncInfo(
                        on_wait=waits[-limit:],
                        on_update=list(si.on_update) if si.on_update else [])
                out.append(ins)
            bb.instructions = out


class _DrainBalancer:
    """Greedy min-load assignment of psum drains / elementwise ops to
    ACT/DVE/Pool using modeled per-op ns."""

    def __init__(self, nc, abias=1.0):
        self.nc = nc
        self.abias = abias
        # pre-charge engines for fixed work emitted outside the balancer
        # (squash chains + vd builds on DVE, misc on ACT/Pool)
        self.load = {"a": 3000.0, "d": 14000.0, "p": 2000.0}

    def _pick(self, costs):
        e = min(costs, key=lambda k: self.load[k] + costs[k])
        self.load[e] += costs[e]
        return e

    def drain(self, dst, src, nelem):
        # GPSIMD cannot access PSUM on hardware: drains are ACT/DVE only.
        # Bias toward ACT, which cannot run tensor_tensor work.
        costs = {"a": (nelem * 0.833 + 143) * self.abias,
                 "d": nelem * 1.042 + 125}
        e = self._pick(costs)
        if e == "a":
            self.nc.scalar.copy(dst, src)
        else:
            self.nc.vector.tensor_copy(dst, src)

    def tt(self, op, out, in0, in1, nelem, x2=False, psum=False):
        """tensor_tensor op on DVE or Pool (ACT cannot; Pool not on psum)."""
        costs = {"d": nelem * 1.042 * (0.5 if x2 else 1.0) + 125,
                 "p": nelem * 1.984 + 95}
        if psum:
            del costs["p"]
        e = self._pick(costs)
        eng = self.nc.vector if e == "d" else self.nc.gpsimd
        getattr(eng, op)(out, in0, in1)


def _ap_pair(sl, off0, stride_t, n):
    """[128, 2, n] AP over a [128, X] slice: t=0 at off0, t=1 at off0+stride_t."""
    ap = sl[:, off0:off0 + n].copy()
    ap.ap = bass_rust.VecI64Pair([list(ap.ap[0]), [stride_t, 2], [1, n]])
    return ap


def build_program(split_waits=True, cfg=None):
    cfg = cfg or {}
    PB = cfg.get('pb', 4)
    PAGA = cfg.get('paga', 1)
    PAGB = cfg.get('pagb', 4)
    PPS1 = cfg.get('pps1', 2)
    PRED = cfg.get('pre_d', 0.0)
    ABIAS = cfg.get('abias', 0.85)
    nc = bass.Bass()
    # --- DRAM parameters ---
    bd_d = [nc.declare_dram_parameter(f"bd{q}", [128, NJ, 2, 128], F8,
                                      isOutput=False) for q in range(NBB)]
    bd0a_d = nc.declare_dram_parameter("bd0a", [128, NJ // 2, 2, 128], F8,
                                       isOutput=False)
    ws8_d = nc.declare_dram_parameter("ws8", [128, NJ, 2, F], F8, isOutput=False)
    wsT1_d = nc.declare_dram_parameter("wsT1", [128, NJ, 2, 128], F8, isOutput=False)
    wsT2_d = nc.declare_dram_parameter("wsT2", [128, NJ, 2, 32], F8, isOutput=False)
    ws16_d = [nc.declare_dram_parameter(f"ws16{h}", [128, 2, NJ // 2, F], BF,
                                        isOutput=False) for h in range(2)]
    xt16_d = nc.declare_dram_parameter("xt16", [128, 2, NJ, B_LOC], BF, isOutput=False)
    id32_d = nc.declare_dram_parameter("id32", [32, 32], BF, isOutput=False)
    mskf_d = nc.declare_dram_parameter("mskf", [40, F], BF, isOutput=False)
    o40x_d = nc.declare_dram_parameter("o40x", [40, NBB, 32], BF, isOutput=False)
    mskS_d = nc.declare_dram_parameter("mskS", [128, 40, NJ], BF, isOutput=False)
    dm0_d = nc.declare_dram_parameter("dm0", [128, 40], BF, isOutput=False)
    dm1_d = nc.declare_dram_parameter("dm1", [32, 40], BF, isOutput=False)
    out_d = nc.declare_dram_parameter("out", [B_LOC, F], F32, isOutput=True)

    with ExitStack() as ctx:
        tc = ctx.enter_context(tile.TileContext(nc))
        st = ctx.enter_context(tc.tile_pool(name="st", bufs=1))
        bdp = ctx.enter_context(tc.tile_pool(name="bdp", bufs=2))
        tmp = ctx.enter_context(tc.tile_pool(name="tmp", bufs=2))
        smp = ctx.enter_context(tc.tile_pool(name="smp", bufs=3))
        mkd = ctx.enter_context(tc.tile_pool(name="mkd", bufs=2))

        # --- persistent SBUF ---
        ws8 = st.tile([128, NJ, 2, F], F8, tag="ws8", name="ws8")
        wsT1 = st.tile([128, NJ, 2, 128], F8, tag="wsT1", name="wsT1")
        wsT2 = st.tile([128, NJ, 2, 32], F8, tag="wsT2", name="wsT2")
        ws16 = st.tile([128, 2, NJ // 2, F], BF, tag="ws16", name="ws16")
        xt16 = st.tile([128, 2, NJ, B_LOC], BF, tag="xt16", name="xt16")
        u8 = st.tile([128, NJ // 2, NBB, 2, F], F8, tag="u8", name="u8")
        uT = st.tile([128, NJQ, NBB, 512], F8, tag="uT", name="uT")
        uT2 = st.tile([32, NJQ, NBB, 4, 128], F8, tag="uT2", name="uT2")
        id32 = st.tile([32, 32], BF, tag="id32", name="id32")
        mskf = st.tile([40, F], BF, tag="mskf", name="mskf")
        o40x = st.tile([40, NBB, 32], BF, tag="o40x", name="o40x")
        mskS = st.tile([128, 40, NJ], BF, tag="mskS", name="mskS")
        dm0 = st.tile([128, 40], BF, tag="dm0", name="dm0")
        dm1 = st.tile([32, 40], BF, tag="dm1", name="dm1")
        s_sb = [st.tile([B_LOC, F], BF, tag=f"s_sb{r}", name=f"s_sb{r}")
                for r in range(2)]
        v32 = [st.tile([B_LOC, F], BF, tag=f"v32_{r}", name=f"v32_{r}")
               for r in range(2)]
        vT1 = st.tile([128, 32], BF, tag="vT1", name="vT1")
        vT2 = st.tile([128, 32], BF, tag="vT2", name="vT2")
        vd = st.tile([128, NBB, 40], F8, tag="vd", name="vd")
        vd2 = st.tile([32, NBB, 40], F8, tag="vd2", name="vd2")
        of32 = st.tile([B_LOC, F], F32, tag="of32", name="of32")
        delta = [st.tile([128, NJ // 2, 2, 48], F8, tag=f"delta{bb}",
                         name=f"delta{bb}")
                 for bb in range(NBB)]

        # --- input loads (HWDGE, no waits) ---
        nc.sync.dma_start(out=xt16[:], in_=xt16_d[:])
        nc.sync.dma_start(out=ws8[:], in_=ws8_d[:])
        nc.sync.dma_start(out=wsT1[:], in_=wsT1_d[:])
        nc.sync.dma_start(out=wsT2[:], in_=wsT2_d[:])
        nc.scalar.dma_start(out=ws16[:], in_=ws16_d[0][:])
        nc.sync.dma_start(out=id32[:], in_=id32_d[:])
        nc.sync.dma_start(out=mskf[:], in_=mskf_d[:])
        nc.sync.dma_start(out=mskS[:], in_=mskS_d[:])
        nc.sync.dma_start(out=o40x[:], in_=o40x_d[:])
        nc.sync.dma_start(out=dm0[:], in_=dm0_d[:])
        nc.sync.dma_start(out=dm1[:], in_=dm1_d[:])
        nc.scalar.memzero(vd[:])
        for bb in range(NBB):
            nc.scalar.memzero(delta[bb][:, :, :, 40:48])

        bal = _DrainBalancer(nc, abias=ABIAS)
        bal.load['d'] = PRED

        def vd_build(vsrc, pvt_pool, pvt_tag):
            """vd[:, bb, 0:44]=t0 / 44*(1+c):...=t1 planes from v [32,160]."""
            pv1 = pvt_pool.tile([128, 32], BF, tag=pvt_tag, name="pv1")
            nc.tensor.transpose(pv1[:], vsrc[:, 0:128], id32[:])
            nc.scalar.activation(vT1[:], pv1[:], AF.Copy, scale=SV)
            pv2 = pvt_pool.tile([128, 32], BF, tag=pvt_tag, name="pv2")
            nc.tensor.transpose(pv2[0:32, :], vsrc[:, 128:160], id32[:])
            nc.scalar.activation(vT2[0:32, :], pv2[0:32, :], AF.Copy, scale=SV)
            # (diag(o) - 0.1) pattern folded into dm0
            in0 = vT1[:].rearrange("p (bb four) -> p bb four", four=4)
            nc.vector.tensor_mul(
                vd[:].rearrange("p bb (four o) -> p bb four o", o=10),
                in0.unsqueeze(-1).broadcast_to((128, NBB, 4, 10)),
                dm0[:].rearrange("p (four o) -> p four o", o=10)
                .unsqueeze(1).broadcast_to((128, NBB, 4, 10)))
            # o in {8,9} plane on 32 partitions
            i2 = vT2[0:32, :].rearrange("p (bb four) -> p bb four", four=4)
            nc.vector.tensor_mul(
                vd2[:].rearrange("p bb (four o) -> p bb four o", o=10),
                i2.unsqueeze(-1).broadcast_to((32, NBB, 4, 10)),
                dm1[:].rearrange("p (four o) -> p four o", o=10)
                .unsqueeze(1).broadcast_to((32, NBB, 4, 10)))

        def agr_jo(bb, jo, pagr):
            """One third of block bb's agreement + its delta mask."""
            if True:
                pag = pagr.tile([128, 12, 40], F32, tag="pag",
                                name=f"pag{bb}_{jo}")
                for jj in range(12):
                    j = 12 * jo + jj
                    jq, jm = j // 4, j % 4
                    nc.tensor.matmul(pag[:, jj, :],
                                     lhsT=uT[:, jq, bb, 128 * jm:128 * (jm + 1)],
                                     rhs=vd[:, bb, :], start=True, stop=False)
                    nc.tensor.matmul(pag[:, jj, :],
                                     lhsT=uT2[:, jq, bb, jm, :],
                                     rhs=vd2[:, bb, :], start=False, stop=True)
                # psum already holds binc - mean_o; GPSIMD can't read psum,
                # so drain (ACT/DVE) then mask (DVE/Pool) from SBUF
                tch = tmp.tile([128, 12, 40], BF, tag="tch", name=f"tch{bb}_{jo}")
                bal.drain(tch[:], pag[:], 480)
                bal.tt("tensor_mul",
                       delta[bb][:, 6 * jo:6 * (jo + 1), :, 0:40]
                       .rearrange("p jp t c -> p (jp t) c"),
                       tch[:], mskS[:, :, 12 * jo:12 * (jo + 1)]
                       .transpose([0, 2, 1]), 480)

        def agr_round(bb, pagr):
            for jo in range(NJQ // 3):
                agr_jo(bb, jo, pagr)

        def stage1(r, bb, pps1, psv, tag="ps1"):
            ps1 = pps1.tile([48, F], F32, tag=tag, name=f"ps1_{r}_{bb}")
            for jp in range(NJ // 2):
                lt = delta[bb][:, jp, :, :]
                nc.tensor.matmul(ps1[:], lhsT=lt, rhs=u8[:, jp, bb, :, :],
                                 start=(jp == 0), stop=(jp == NJ // 2 - 1),
                                 perf_mode=DR)
            md = mkd.tile([40, F], BF, tag="mkd", name=f"mkd_{r}_{bb}")
            bal.tt("tensor_mul", md[:], ps1[0:40, :], mskf[:], F, psum=True)
            nc.tensor.matmul(psv[:], lhsT=o40x[:, bb, :], rhs=md[:],
                             start=False, stop=(bb == NBB - 1))
            return ps1

        # ================= phase A: builds + iter-0 =================
        with tc.tile_pool(name="pb", bufs=PB, space="PSUM") as pb, \
             tc.tile_pool(name="pT2", bufs=2, space="PSUM") as pT2, \
             tc.tile_pool(name="pagrA", bufs=PAGA, space="PSUM") as pagrA, \
             tc.tile_pool(name="ppsvA", bufs=1, space="PSUM") as ppsvA:
            psv1 = ppsvA.tile([B_LOC, F], F32, tag="psv1", name="psv1")
            # pass-1 (fp16): ps0 = SW * sum_i u_hat  -> s0, v0
            # (two j-halves around q0's build so PE fills the load gap;
            #  ps0 borrows the pagrA slot, released before agr0 needs it)
            ps0 = pagrA.tile([B_LOC, F], F32, tag="pag", name="ps0")

            def pass1_half(h):
                for j in range(h * NJ // 2, (h + 1) * NJ // 2):
                    for t in range(2):
                        nc.tensor.matmul(ps0[:], lhsT=xt16[:, t, j, :],
                                         rhs=ws16[:, t, j % (NJ // 2), :],
                                         start=(j == 0 and t == 0),
                                         stop=(j == NJ - 1 and t == 1))
            pass1_half(0)
            pend = []

            def pop_pend(n):
                for _ in range(n):
                    if pend:
                        pend.pop(0)()

            for bb in range(NBB):
                bdt = bdp.tile([128, NJ, 2, 128], F8, tag="bdt", name=f"bdt{bb}")
                # ring-slot reuse gives blocks >=2 multiple waits -> SWDGE
                eng = (nc.scalar, nc.sync, nc.gpsimd, nc.gpsimd, nc.gpsimd,
                       nc.gpsimd, nc.gpsimd, nc.gpsimd)[bb]
                if bb == 0:
                    # split first block's load so early builds start sooner
                    eng.dma_start(out=bdt[:, 0:NJ // 2], in_=bd0a_d[:])
                    eng.dma_start(out=bdt[:, NJ // 2:],
                                  in_=bd_d[0][:, NJ // 2:])
                    nc.gpsimd.dma_start(out=ws16[:], in_=ws16_d[1][:])
                else:
                    eng.dma_start(out=bdt[:], in_=bd_d[bb][:])
                if True:
                    for jq in range(NJQ):
                        p2 = pT2.tile([32, 4, 128], F32, tag="p2",
                                      name=f"p2_{bb}_{jq}")
                        p1 = pb.tile([128, 4, 128], F32, tag="pb", bufs=PB,
                                     name=f"p1{jq}_{bb}")
                        for jh in range(2):
                            jp = 2 * jq + jh
                            pu = pb.tile([128, 2, F], F32, tag="pb", bufs=PB,
                                         name=f"pu{jp}_{bb}")
                            for jm2 in range(2):
                                j = 2 * jp + jm2
                                jm = 2 * jh + jm2
                                bsl = bdt[:, j, :, :]
                                nc.tensor.matmul(pu[:, jm2, :], lhsT=bsl,
                                                 rhs=ws8[:, j, :, :],
                                                 start=True, stop=True,
                                                 perf_mode=DR)
                                nc.tensor.matmul(p1[:, jm, :],
                                                 lhsT=wsT1[:, j, :, :],
                                                 rhs=bsl, start=True,
                                                 stop=True, perf_mode=DR)
                                nc.tensor.matmul(p2[:, jm, :],
                                                 lhsT=wsT2[:, j, :, :],
                                                 rhs=bsl, start=True,
                                                 stop=True, perf_mode=DR)
                            bal.drain(u8[:, jp, bb, :, :], pu[:], 2 * F)
                        bal.drain(uT[:, jq, bb, 0:512], p1[:], 512)
                        bal.drain(uT2[:, jq, bb, :, :], p2[:], 512)
                        if bb > 0:
                            pop_pend(1)
                if bb == 0:
                    pass1_half(1)
                    _squash(nc, smp, ps0, 0.1 / SW, v32[0])
                    nc.scalar.activation(s_sb[0][:], ps0[:], AF.Copy,
                                         scale=0.1 / SW)
                    nc.tensor.matmul(psv1[:], lhsT=id32[:], rhs=s_sb[0][:],
                                     start=True, stop=False)
                    vd_build(v32[0], pagrA, "pag")
                for jo in range(NJQ // 3):
                    pend.append(lambda b=bb, o=jo: agr_jo(b, o, pagrA))
                pend.append(lambda b=bb: stage1(1, b, pagrA, psv1, tag="pag"))
            pop_pend(len(pend))
            _squash(nc, smp, psv1, 1.0, v32[1])
            nc.scalar.copy(s_sb[1][:], psv1[:])

        # ================= phase B: routing rounds =================
        with tc.tile_pool(name="pagrB", bufs=PAGB, space="PSUM") as pagrB, \
             tc.tile_pool(name="pps1", bufs=PPS1, space="PSUM") as pps1, \
             tc.tile_pool(name="ppsv", bufs=1, space="PSUM") as ppsv, \
             tc.tile_pool(name="pvt", bufs=1, space="PSUM") as pvt:
            vd_build(v32[1], pvt, "pvt")
            psv = ppsv.tile([B_LOC, F], F32, tag="psv", name="psv2")
            nc.tensor.matmul(psv[:], lhsT=id32[:], rhs=s_sb[1][:],
                             start=True, stop=False)
            for bb in range(NBB):
                agr_round(bb, pagrB)
                stage1(2, bb, pps1, psv)
            _squash(nc, smp, psv, 1.0, of32)
            nc.gpsimd.dma_start(out=out_d[:], in_=of32[:])

    if split_waits:
        _split_multiwait(nc)
    return nc


def _host_inputs(x, W):
    """Per-core input maps from full x [256,1152,8] f32, W [1,1152,10,16,8]."""
    f8 = mybir.dt.np(F8)
    bf = np.float16
    W0 = np.asarray(W[0], dtype=np.float32) * SW
    # ws[q=(ii16,k8), t, j, (o,d)] = W[j*32+t*16+ii, o, d, k] * SW
    Wr = W0.reshape(NJ, 2, 16, OC, OD, ID)
    ws = np.ascontiguousarray(
        Wr.transpose(2, 5, 1, 0, 3, 4).reshape(128, 2, NJ, F))
    ws16 = ws.astype(bf)
    wsj = np.ascontiguousarray(ws.transpose(0, 2, 1, 3))   # [128, NJ, 2, F]
    ws8 = wsj.astype(f8)
    wsT1 = np.ascontiguousarray(wsj[:, :, :, 0:128]).astype(f8)
    wsT2 = np.ascontiguousarray(wsj[:, :, :, 128:160]).astype(f8)
    id32 = np.eye(32, dtype=bf)
    mskf = np.zeros((40, F), dtype=bf)
    for bp in range(4):
        for o in range(OC):
            mskf[bp * 10 + o, o * OD:(o + 1) * OD] = 1.0
    o40x = np.zeros((40, NBB, 32), dtype=bf)
    for bb in range(NBB):
        for bp in range(4):
            for o in range(OC):
                o40x[bp * 10 + o, bb, bb * 4 + bp] = 1.0 / (SW * DSC)
    mskS = np.zeros((128, 40, NJ), dtype=bf)
    for p in range(128):
        bp = p // 32
        mskS[p, bp * 10:(bp + 1) * 10, :] = 0.1 * DSC / PSA
    dm0 = np.zeros((128, 40), dtype=bf)
    for p in range(128):
        o = p // 16
        for bp in range(4):
            for o2 in range(OC):
                dm0[p, bp * 10 + o2] = (1.0 if o2 == o else 0.0) - 0.1
    dm1 = np.zeros((32, 40), dtype=bf)
    for pp in range(32):
        o = 8 + pp // 16
        for bp in range(4):
            for o2 in range(OC):
                dm1[pp, bp * 10 + o2] = (1.0 if o2 == o else 0.0) - 0.1

    in_maps = []
    for core in range(N_CORES):
        xc = np.asarray(x[core * B_LOC:(core + 1) * B_LOC], dtype=np.float32)
        # xt16[q=(ii,k), t, j, b] = x[b, j*32+t*16+ii, k]
        xr = xc.reshape(B_LOC, NJ, 2, 16, ID)
        xt16 = np.ascontiguousarray(
            xr.transpose(3, 4, 2, 1, 0).reshape(128, 2, NJ, B_LOC)).astype(bf)
        # bd[q][K=(ii16,k8), j, bi, t, m=(bp,ii32)] =
        #   x[(2q+bi)*4+bp, j*32+ii32, k] * (ii32 == t*16+ii16)
        x8 = xc.astype(f8).astype(np.float32)
        bds = []
        for bb in range(NBB):
            xq = x8[4 * bb:4 * (bb + 1)].reshape(4, NJ, 2, 16, ID)
            z = np.zeros((16, ID, NJ, 2, 4, 32), dtype=np.float32)
            for t in range(2):
                for ii in range(16):
                    # [bp, j, k] -> [k, j, bp]
                    z[ii, :, :, t, :, t * 16 + ii] = (
                        xq[:, :, t, ii, :].transpose(2, 1, 0))
            bds.append(np.ascontiguousarray(
                z.reshape(128, NJ, 2, 128)).astype(f8))
        m = {"ws8": ws8, "wsT1": wsT1, "wsT2": wsT2,
             "ws160": ws16[:, :, 0:NJ // 2, :].copy(),
             "ws161": ws16[:, :, NJ // 2:, :].copy(), "xt16": xt16,
             "id32": id32, "mskf": mskf, "o40x": o40x, "mskS": mskS,
             "dm0": dm0, "dm1": dm1}
        for q in range(NBB):
            m[f"bd{q}"] = bds[q]
        m["bd0a"] = bds[0][:, 0:NJ // 2].copy()
        in_maps.append(m)
    return in_maps


_NC_CACHE = {}


def kernel(x, W):
    from concourse.bass_utils import run_bass_kernel_spmd

    if "nc" not in _NC_CACHE:
        _NC_CACHE["nc"] = build_program()
    nc = _NC_CACHE["nc"]
    in_maps = _host_inputs(x, W)
    res = run_bass_kernel_spmd(nc, in_maps, core_ids=list(range(N_CORES)))
    out = np.concatenate([r["out"] for r in res.results], axis=0)
    return out.reshape(B_FULL, OC, OD).astype(np.float32)


if __name__ == "__main__":
    nc = build_program()
    print("program built ok,",
          sum(len(b.instructions) for f in nc.m.functions for b in f.blocks),
          "instructions")


# revision 4
# speedup vs baseline: 1.2218x; 1.0309x over previous
"""Trainium2 Bass kernel for the CapsuleNet dynamic-routing layer, v2.

Math (per batch element b):
    u_hat[b,i,o,d] = sum_k W[i,o,d,k] * x[b,i,k]   # B=256, IC=1152, OC=10, OD=16, ID=8
    b_log = 0
    for it in 0..2:
        c = softmax(b_log, axis=o); s = sum_i c*u_hat; v = squash(s)
        if it < 2: b_log += sum_d u_hat * v

v2 design (vs the v1 DVE-agreement kernel):
  * Data-parallel over B across 8 cores (32 local rows), W replicated.
  * Partition layout p=(bp4, ii32): b-blocks of 4 (bb in 0..8), i-blocks of 32
    (j in 0..36).  All heavy contractions are fp8e4 DoubleRow matmuls on PE.
  * s is split s = s0 + delta-terms: s0 = 0.1*sum_i u_hat comes from an fp16
    pass-1 (x16 @ W16, full precision); everything delta-scaled (0.4% of s)
    runs in fp8 (error-tolerant).
  * The agreement (b_inc = sum_d u_hat*v) moves from DVE onto the PE via a
    transposed fp8 copy u_hatT[(o,d), (bp,ii)] built directly by ws8 x bd
    matmuls.  Agreement output lands as psum [128,(bp',o')=40+4] per (j,bb)
    whose 40 columns ARE the c-blockdiag positions for stage-1: no DRAM
    bounces at all.  4 extra M-columns compute sum_o b_inc/10 in the same
    matmul, so softmax is LINEARIZED (|logits|~4e-3): delta = (b_inc -
    mean_o)*0.1, computed by one DVE subtract + one mask-multiply.
  * Stage-1 s-increments accumulate across routing iterations in a [32,160]
    psum (delta_t are increments); one squash per round.
  * o in 10 = 8+2: u_hatT keeps (o<8,d) on 128 partitions (plane t0) and
    (o>=8,d) 4-up packed over j%4 (plane t1); the agreement DoubleRow matmul
    sums both planes with per-j variable-stride APs and per-class vd masks.
"""

import sys

sys.path.insert(0, "/opt/trn_rl_repo")

from contextlib import ExitStack

import numpy as np

import bass_rust
import concourse.bass as bass
import concourse.tile as tile
from concourse import mybir

BF = mybir.dt.float16
F8 = mybir.dt.float8e4
F32 = mybir.dt.float32
AX = mybir.AxisListType
AF = mybir.ActivationFunctionType
DR = mybir.MatmulPerfMode.DoubleRow

N_CORES = 8
B_FULL, IC, OC, OD, ID = 256, 1152, 10, 16, 8
B_LOC = B_FULL // N_CORES            # 32
F = OC * OD                          # 160
NJ = 36                              # i-blocks of 32
NJQ = 9                              # jq groups of 4 j-blocks
NBB = 8                              # b-blocks of 4
SW = 64.0                            # W scale into fp8
SV = 32.0                            # v scale into fp8
PSA = 2048.0                         # agreement psum scale = SW*SV
DSC = 1024.0                         # delta fp8 scale


def _squash(nc, smp, ps, scale, vout, P=B_LOC):
    """vout = squash(scale * ps) with ps a [P, 160] psum slab (f32).

    squash(s) = n2/((1+n2)(n+eps)) * s with n2 = |s|^2 per (b,o).
    """
    sq = smp.tile([P, F], F32, tag=f"sq{P}", name="sq")
    nc.scalar.activation(sq[:], ps[:], AF.Square, scale=float(scale))
    n2 = smp.tile([P, OC], F32, tag=f"n2{P}", name="n2")
    nc.vector.tensor_reduce(
        n2[:], sq[:].rearrange("p (o d) -> p o d", d=OD), axis=AX.X,
        op=mybir.AluOpType.add)
    n1 = smp.tile([P, OC], F32, tag=f"n1{P}", name="n1")
    nc.scalar.add(n1[:], n2[:], 1.0)
    sn = smp.tile([P, OC], F32, tag=f"sn{P}", name="sn")
    nc.scalar.sqrt(sn[:], n2[:])
    t1 = smp.tile([P, OC], F32, tag=f"t1{P}", name="t1")
    nc.vector.tensor_mul(t1[:], n1[:], sn[:])
    r1 = smp.tile([P, OC], F32, tag=f"r1{P}", name="r1")
    nc.vector.reciprocal(r1[:], t1[:])
    f1 = smp.tile([P, OC], F32, tag=f"f1{P}", name="f1")
    nc.vector.tensor_mul(f1[:], n2[:], r1[:])
    if scale != 1.0:
        nc.scalar.mul(f1[:], f1[:], float(scale))
    nc.vector.tensor_mul(
        vout[:].rearrange("p (o d) -> p o d", d=OD),
        ps[:].rearrange("p (o d) -> p o d", d=OD),
        f1[:].unsqueeze(-1).broadcast_to((P, OC, OD)))


def _split_multiwait(nc):
    """Walrus encodes at most ONE semaphore wait on Matmult/Ldweights and
    HWDGE DMACopy instructions.  Hoist excess waits onto same-engine NoOps."""
    for fn in nc.m.functions:
        for bb in fn.blocks:
            out = []
            k = 0
            for ins in bb.instructions:
                si = ins.sync_info
                waits = list(si.on_wait) if si is not None and si.on_wait else []
                limit = 1
                if ins.opcode == "DMACopy":
                    q = str(getattr(ins, "queue", "") or "")
                    if "HW" in q and len(waits) > 1:
                        raise AssertionError(
                            f"HWDGE DMA {ins.name} has {len(waits)} waits: {ins}")
                if len(waits) > limit:
                    for w in waits[:-limit]:
                        nop = mybir.InstNoOp(name=f"{ins.name}-wn{k}", ins=[], outs=[])
                        k += 1
                        nop.engine = ins.engine
                        nop.sync_info = mybir.SyncInfo(on_wait=[w], on_update=[])
                        out.append(nop)
                    ins.sync_info = mybir.SyncInfo(
                        on_wait=waits[-limit:],
                        on_update=list(si.on_update) if si.on_update else [])
                out.append(ins)
            bb.instructions = out


class _DrainBalancer:
    """Greedy min-load assignment of psum drains / elementwise ops to
    ACT/DVE/Pool using modeled per-op ns."""

    def __init__(self, nc, abias=1.0):
        self.nc = nc
        self.abias = abias
        self.pbias = 1.0
        # pre-charge engines for fixed work emitted outside the balancer
        # (squash chains + vd builds on DVE, misc on ACT/Pool)
        self.load = {"a": 3000.0, "d": 14000.0, "p": 2000.0}

    def _pick(self, costs):
        e = min(costs, key=lambda k: self.load[k] + costs[k])
        self.load[e] += costs[e]
        return e

    def drain(self, dst, src, nelem):
        # GPSIMD cannot access PSUM on hardware: drains are ACT/DVE only.
        # Bias toward ACT, which cannot run tensor_tensor work.
        costs = {"a": (nelem * 0.833 + 143) * self.abias,
                 "d": nelem * 1.042 + 125}
        e = self._pick(costs)
        if e == "a":
            self.nc.scalar.copy(dst, src)
        else:
            self.nc.vector.tensor_copy(dst, src)

    def tt(self, op, out, in0, in1, nelem, x2=False, psum=False):
        """tensor_tensor op on DVE or Pool (ACT cannot; Pool not on psum)."""
        costs = {"d": nelem * 1.042 * (0.5 if x2 else 1.0) + 125,
                 "p": (nelem * 1.984 + 95) * self.pbias}
        if psum:
            del costs["p"]
        e = self._pick(costs)
        eng = self.nc.vector if e == "d" else self.nc.gpsimd
        getattr(eng, op)(out, in0, in1)


def _ap_pair(sl, off0, stride_t, n):
    """[128, 2, n] AP over a [128, X] slice: t=0 at off0, t=1 at off0+stride_t."""
    ap = sl[:, off0:off0 + n].copy()
    ap.ap = bass_rust.VecI64Pair([list(ap.ap[0]), [stride_t, 2], [1, n]])
    return ap


def build_program(split_waits=True, cfg=None):
    cfg = cfg or {}
    PB = cfg.get('pb', 4)
    PAGA = cfg.get('paga', 1)
    PT2 = cfg.get('pt2', 2)
    PAGB = cfg.get('pagb', 4)
    PPS1 = cfg.get('pps1', 2)
    PRED = cfg.get('pre_d', 0.0)
    ABIAS = cfg.get('abias', 1.15)
    PBA = cfg.get('pb_a', 0.55)
    PBB = cfg.get('pb_b', 1.8)
    nc = bass.Bass()
    # --- DRAM parameters ---
    bd_d = [nc.declare_dram_parameter(f"bd{q}", [128, NJ, 2, 128], F8,
                                      isOutput=False) for q in range(NBB)]
    bd0a_d = nc.declare_dram_parameter("bd0a", [128, NJ // 2, 2, 128], F8,
                                       isOutput=False)
    ws8_d = nc.declare_dram_parameter("ws8", [128, NJ, 2, F], F8, isOutput=False)
    wsT1_d = nc.declare_dram_parameter("wsT1", [128, NJ, 2, 128], F8, isOutput=False)
    wsT2_d = nc.declare_dram_parameter("wsT2", [128, NJ, 2, 32], F8, isOutput=False)
    ws16_d = [nc.declare_dram_parameter(f"ws16{h}", [128, 2, NJ // 2, F], BF,
                                        isOutput=False) for h in range(2)]
    xt16_d = nc.declare_dram_parameter("xt16", [128, 2, NJ, B_LOC], BF, isOutput=False)
    id32_d = nc.declare_dram_parameter("id32", [32, 32], BF, isOutput=False)
    mskf_d = nc.declare_dram_parameter("mskf", [40, F], BF, isOutput=False)
    o40x_d = nc.declare_dram_parameter("o40x", [40, NBB, 32], BF, isOutput=False)
    mskS_d = nc.declare_dram_parameter("mskS", [128, 40, NJ], BF, isOutput=False)
    dm0_d = nc.declare_dram_parameter("dm0", [128, 40], BF, isOutput=False)
    dm1_d = nc.declare_dram_parameter("dm1", [32, 40], BF, isOutput=False)
    out_d = nc.declare_dram_parameter("out", [B_LOC, F], F32, isOutput=True)

    with ExitStack() as ctx:
        tc = ctx.enter_context(tile.TileContext(nc))
        st = ctx.enter_context(tc.tile_pool(name="st", bufs=1))
        bdp = ctx.enter_context(tc.tile_pool(name="bdp", bufs=2))
        tmp = ctx.enter_context(tc.tile_pool(name="tmp", bufs=2))
        smp = ctx.enter_context(tc.tile_pool(name="smp", bufs=4))
        mkd = ctx.enter_context(tc.tile_pool(name="mkd", bufs=2))

        # --- persistent SBUF ---
        ws8 = st.tile([128, NJ, 2, F], F8, tag="ws8", name="ws8")
        wsT1 = st.tile([128, NJ, 2, 128], F8, tag="wsT1", name="wsT1")
        wsT2 = st.tile([128, NJ, 2, 32], F8, tag="wsT2", name="wsT2")
        ws16 = st.tile([128, 2, NJ // 2, F], BF, tag="ws16", name="ws16")
        xt16 = st.tile([128, 2, NJ, B_LOC], BF, tag="xt16", name="xt16")
        u8 = st.tile([128, NJ // 2, NBB, 2, F], F8, tag="u8", name="u8")
        uT = st.tile([128, NJQ, NBB, 512], F8, tag="uT", name="uT")
        uT2 = st.tile([32, NJQ, NBB, 4, 128], F8, tag="uT2", name="uT2")
        id32 = st.tile([32, 32], BF, tag="id32", name="id32")
        mskf = st.tile([40, F], BF, tag="mskf", name="mskf")
        o40x = st.tile([40, NBB, 32], BF, tag="o40x", name="o40x")
        mskS = st.tile([128, 40, NJ], BF, tag="mskS", name="mskS")
        dm0 = st.tile([128, 40], BF, tag="dm0", name="dm0")
        dm1 = st.tile([32, 40], BF, tag="dm1", name="dm1")
        s_sb = [st.tile([B_LOC, F], BF, tag=f"s_sb{r}", name=f"s_sb{r}")
                for r in range(2)]
        v32 = [st.tile([B_LOC, F], BF, tag=f"v32_{r}", name=f"v32_{r}")
               for r in range(2)]
        vT1 = st.tile([128, 32], BF, tag="vT1", name="vT1")
        vT2 = st.tile([128, 32], BF, tag="vT2", name="vT2")
        vd = st.tile([128, NBB, 40], F8, tag="vd", name="vd")
        vd2 = st.tile([32, NBB, 40], F8, tag="vd2", name="vd2")
        of32 = st.tile([B_LOC, F], F32, tag="of32", name="of32")
        delta = [st.tile([128, NJ // 2, 2, 48], F8, tag=f"delta{bb}",
                         name=f"delta{bb}")
                 for bb in range(NBB)]

        # --- input loads (HWDGE, no waits) ---
        nc.sync.dma_start(out=xt16[:], in_=xt16_d[:])
        nc.sync.dma_start(out=ws8[:], in_=ws8_d[:])
        nc.sync.dma_start(out=wsT1[:], in_=wsT1_d[:])
        nc.sync.dma_start(out=wsT2[:], in_=wsT2_d[:])
        nc.scalar.dma_start(out=ws16[:], in_=ws16_d[0][:])
        nc.sync.dma_start(out=id32[:], in_=id32_d[:])
        nc.sync.dma_start(out=mskf[:], in_=mskf_d[:])
        nc.sync.dma_start(out=mskS[:], in_=mskS_d[:])
        nc.sync.dma_start(out=o40x[:], in_=o40x_d[:])
        nc.sync.dma_start(out=dm0[:], in_=dm0_d[:])
        nc.sync.dma_start(out=dm1[:], in_=dm1_d[:])
        nc.scalar.memzero(vd[:])
        for bb in range(NBB):
            nc.scalar.memzero(delta[bb][:, :, :, 40:48])

        bal = _DrainBalancer(nc, abias=ABIAS)
        bal.pbias = PBA
        bal.load['d'] = PRED

        def vd_build(vsrc, pvt_pool, pvt_tag):
            """vd[:, bb, 0:44]=t0 / 44*(1+c):...=t1 planes from v [32,160]."""
            pv1 = pvt_pool.tile([128, 32], BF, tag=pvt_tag, name="pv1")
            nc.tensor.transpose(pv1[:], vsrc[:, 0:128], id32[:])
            nc.scalar.activation(vT1[:], pv1[:], AF.Copy, scale=SV)
            pv2 = pvt_pool.tile([128, 32], BF, tag=pvt_tag, name="pv2")
            nc.tensor.transpose(pv2[0:32, :], vsrc[:, 128:160], id32[:])
            nc.scalar.activation(vT2[0:32, :], pv2[0:32, :], AF.Copy, scale=SV)
            # (diag(o) - 0.1) pattern folded into dm0
            in0 = vT1[:].rearrange("p (bb four) -> p bb four", four=4)
            nc.vector.tensor_mul(
                vd[:].rearrange("p bb (four o) -> p bb four o", o=10),
                in0.unsqueeze(-1).broadcast_to((128, NBB, 4, 10)),
                dm0[:].rearrange("p (four o) -> p four o", o=10)
                .unsqueeze(1).broadcast_to((128, NBB, 4, 10)))
            # o in {8,9} plane on 32 partitions
            i2 = vT2[0:32, :].rearrange("p (bb four) -> p bb four", four=4)
            nc.vector.tensor_mul(
                vd2[:].rearrange("p bb (four o) -> p bb four o", o=10),
                i2.unsqueeze(-1).broadcast_to((32, NBB, 4, 10)),
                dm1[:].rearrange("p (four o) -> p four o", o=10)
                .unsqueeze(1).broadcast_to((32, NBB, 4, 10)))

        def agr_jo(bb, jo, pagr):
            """One third of block bb's agreement + its delta mask."""
            if True:
                pag = pagr.tile([128, 12, 40], F32, tag="pag",
                                name=f"pag{bb}_{jo}")
                for jj in range(12):
                    j = 12 * jo + jj
                    jq, jm = j // 4, j % 4
                    nc.tensor.matmul(pag[:, jj, :],
                                     lhsT=uT[:, jq, bb, 128 * jm:128 * (jm + 1)],
                                     rhs=vd[:, bb, :], start=True, stop=False)
                    nc.tensor.matmul(pag[:, jj, :],
                                     lhsT=uT2[:, jq, bb, jm, :],
                                     rhs=vd2[:, bb, :], start=False, stop=True)
                # psum already holds binc - mean_o; GPSIMD can't read psum,
                # so drain (ACT/DVE) then mask (DVE/Pool) from SBUF
                tch = tmp.tile([128, 12, 40], BF, tag="tch", name=f"tch{bb}_{jo}")
                bal.drain(tch[:], pag[:], 480)
                bal.tt("tensor_mul",
                       delta[bb][:, 6 * jo:6 * (jo + 1), :, 0:40]
                       .rearrange("p jp t c -> p (jp t) c"),
                       tch[:], mskS[:, :, 12 * jo:12 * (jo + 1)]
                       .transpose([0, 2, 1]), 480)

        def agr_round(bb, pagr):
            for jo in range(NJQ // 3):
                agr_jo(bb, jo, pagr)

        def stage1(r, bb, pps1, psv, tag="ps1"):
            ps1 = pps1.tile([48, F], F32, tag=tag, name=f"ps1_{r}_{bb}")
            for jp in range(NJ // 2):
                lt = delta[bb][:, jp, :, :]
                nc.tensor.matmul(ps1[:], lhsT=lt, rhs=u8[:, jp, bb, :, :],
                                 start=(jp == 0), stop=(jp == NJ // 2 - 1),
                                 perf_mode=DR)
            md = mkd.tile([40, F], BF, tag="mkd", name=f"mkd_{r}_{bb}")
            bal.tt("tensor_mul", md[:], ps1[0:40, :], mskf[:], F, psum=True)
            nc.tensor.matmul(psv[:], lhsT=o40x[:, bb, :], rhs=md[:],
                             start=False, stop=(bb == NBB - 1))
            return ps1

        # ================= phase A: builds + iter-0 =================
        with tc.tile_pool(name="pb", bufs=PB, space="PSUM") as pb, \
             tc.tile_pool(name="pT2", bufs=PT2, space="PSUM") as pT2, \
             tc.tile_pool(name="pagrA", bufs=PAGA, space="PSUM") as pagrA, \
             tc.tile_pool(name="ppsvA", bufs=1, space="PSUM") as ppsvA:
            psv1 = ppsvA.tile([B_LOC, F], F32, tag="psv1", name="psv1")
            # pass-1 (fp16): ps0 = SW * sum_i u_hat  -> s0, v0
            # (two j-halves around q0's build so PE fills the load gap;
            #  ps0 borrows the pagrA slot, released before agr0 needs it)
            ps0 = pagrA.tile([B_LOC, F], F32, tag="pag", name="ps0")

            def pass1_half(h):
                for j in range(h * NJ // 2, (h + 1) * NJ // 2):
                    for t in range(2):
                        nc.tensor.matmul(ps0[:], lhsT=xt16[:, t, j, :],
                                         rhs=ws16[:, t, j % (NJ // 2), :],
                                         start=(j == 0 and t == 0),
                                         stop=(j == NJ - 1 and t == 1))
            pass1_half(0)
            pend = []

            def pop_pend(n):
                for _ in range(n):
                    if pend:
                        pend.pop(0)()

            for bb in range(NBB):
                bdt = bdp.tile([128, NJ, 2, 128], F8, tag="bdt", name=f"bdt{bb}")
                # ring-slot reuse gives blocks >=2 multiple waits -> SWDGE
                eng = (nc.scalar, nc.sync, nc.gpsimd, nc.gpsimd, nc.gpsimd,
                       nc.gpsimd, nc.gpsimd, nc.gpsimd)[bb]
                if bb == 0:
                    # split first block's load so early builds start sooner
                    eng.dma_start(out=bdt[:, 0:NJ // 2], in_=bd0a_d[:])
                    eng.dma_start(out=bdt[:, NJ // 2:],
                                  in_=bd_d[0][:, NJ // 2:])
                    nc.gpsimd.dma_start(out=ws16[:], in_=ws16_d[1][:])
                else:
                    eng.dma_start(out=bdt[:], in_=bd_d[bb][:])
                if True:
                    for jq in range(NJQ):
                        p2 = pT2.tile([32, 4, 128], F32, tag="p2",
                                      name=f"p2_{bb}_{jq}")
                        p1 = pb.tile([128, 4, 128], F32, tag="pb", bufs=PB,
                                     name=f"p1{jq}_{bb}")
                        for jh in range(2):
                            jp = 2 * jq + jh
                            pu = pb.tile([128, 2, F], F32, tag="pb", bufs=PB,
                                         name=f"pu{jp}_{bb}")
                            for jm2 in range(2):
                                j = 2 * jp + jm2
                                jm = 2 * jh + jm2
                                bsl = bdt[:, j, :, :]
                                nc.tensor.matmul(pu[:, jm2, :], lhsT=bsl,
                                                 rhs=ws8[:, j, :, :],
                                                 start=True, stop=True,
                                                 perf_mode=DR)
                                nc.tensor.matmul(p1[:, jm, :],
                                                 lhsT=wsT1[:, j, :, :],
                                                 rhs=bsl, start=True,
                                                 stop=True, perf_mode=DR)
                                nc.tensor.matmul(p2[:, jm, :],
                                                 lhsT=wsT2[:, j, :, :],
                                                 rhs=bsl, start=True,
                                                 stop=True, perf_mode=DR)
                            bal.drain(u8[:, jp, bb, :, :], pu[:], 2 * F)
                        bal.drain(uT[:, jq, bb, 0:512], p1[:], 512)
                        bal.drain(uT2[:, jq, bb, :, :], p2[:], 512)
                        if bb > 0:
                            pop_pend(1)
                if bb == 0:
                    pass1_half(1)
                    _squash(nc, smp, ps0, 0.1 / SW, v32[0])
                    nc.scalar.activation(s_sb[0][:], ps0[:], AF.Copy,
                                         scale=0.1 / SW)
                    nc.tensor.matmul(psv1[:], lhsT=id32[:], rhs=s_sb[0][:],
                                     start=True, stop=False)
                    vd_build(v32[0], pagrA, "pag")
                for jo in range(NJQ // 3):
                    pend.append(lambda b=bb, o=jo: agr_jo(b, o, pagrA))
                pend.append(lambda b=bb: stage1(1, b, pagrA, psv1, tag="pag"))
            pop_pend(len(pend))
            bal.pbias = PBB
            _squash(nc, smp, psv1, 1.0, v32[1])
            nc.scalar.copy(s_sb[1][:], psv1[:])

        # ================= phase B: routing rounds =================
        with tc.tile_pool(name="pagrB", bufs=PAGB, space="PSUM") as pagrB, \
             tc.tile_pool(name="pps1", bufs=PPS1, space="PSUM") as pps1, \
             tc.tile_pool(name="ppsv", bufs=1, space="PSUM") as ppsv, \
             tc.tile_pool(name="pvt", bufs=1, space="PSUM") as pvt:
            vd_build(v32[1], pvt, "pvt")
            psv = ppsv.tile([B_LOC, F], F32, tag="psv", name="psv2")
            nc.tensor.matmul(psv[:], lhsT=id32[:], rhs=s_sb[1][:],
                             start=True, stop=False)
            for bb in range(NBB):
                agr_round(bb, pagrB)
                stage1(2, bb, pps1, psv)
            _squash(nc, smp, psv, 1.0, of32)
            nc.gpsimd.dma_start(out=out_d[:], in_=of32[:])

    if split_waits:
        _split_multiwait(nc)
    return nc


def _host_inputs(x, W):
    """Per-core input maps from full x [256,1152,8] f32, W [1,1152,10,16,8]."""
    f8 = mybir.dt.np(F8)
    bf = np.float16
    W0 = np.asarray(W[0], dtype=np.float32) * SW
    # ws[q=(ii16,k8), t, j, (o,d)] = W[j*32+t*16+ii, o, d, k] * SW
    Wr = W0.reshape(NJ, 2, 16, OC, OD, ID)
    ws = np.ascontiguousarray(
        Wr.transpose(2, 5, 1, 0, 3, 4).reshape(128, 2, NJ, F))
    ws16 = ws.astype(bf)
    wsj = np.ascontiguousarray(ws.transpose(0, 2, 1, 3))   # [128, NJ, 2, F]
    ws8 = wsj.astype(f8)
    wsT1 = np.ascontiguousarray(wsj[:, :, :, 0:128]).astype(f8)
    wsT2 = np.ascontiguousarray(wsj[:, :, :, 128:160]).astype(f8)
    id32 = np.eye(32, dtype=bf)
    mskf = np.zeros((40, F), dtype=bf)
    for bp in range(4):
        for o in range(OC):
            mskf[bp * 10 + o, o * OD:(o + 1) * OD] = 1.0
    o40x = np.zeros((40, NBB, 32), dtype=bf)
    for bb in range(NBB):
        for bp in range(4):
            for o in range(OC):
                o40x[bp * 10 + o, bb, bb * 4 + bp] = 1.0 / (SW * DSC)
    mskS = np.zeros((128, 40, NJ), dtype=bf)
    for p in range(128):
        bp = p // 32
        mskS[p, bp * 10:(bp + 1) * 10, :] = 0.1 * DSC / PSA
    dm0 = np.zeros((128, 40), dtype=bf)
    for p in range(128):
        o = p // 16
        for bp in range(4):
            for o2 in range(OC):
                dm0[p, bp * 10 + o2] = (1.0 if o2 == o else 0.0) - 0.1
    dm1 = np.zeros((32, 40), dtype=bf)
    for pp in range(32):
        o = 8 + pp // 16
        for bp in range(4):
            for o2 in range(OC):
                dm1[pp, bp * 10 + o2] = (1.0 if o2 == o else 0.0) - 0.1

    in_maps = []
    for core in range(N_CORES):
        xc = np.asarray(x[core * B_LOC:(core + 1) * B_LOC], dtype=np.float32)
        # xt16[q=(ii,k), t, j, b] = x[b, j*32+t*16+ii, k]
        xr = xc.reshape(B_LOC, NJ, 2, 16, ID)
        xt16 = np.ascontiguousarray(
            xr.transpose(3, 4, 2, 1, 0).reshape(128, 2, NJ, B_LOC)).astype(bf)
        # bd[q][K=(ii16,k8), j, bi, t, m=(bp,ii32)] =
        #   x[(2q+bi)*4+bp, j*32+ii32, k] * (ii32 == t*16+ii16)
        x8 = xc.astype(f8).astype(np.float32)
        bds = []
        for bb in range(NBB):
            xq = x8[4 * bb:4 * (bb + 1)].reshape(4, NJ, 2, 16, ID)
            z = np.zeros((16, ID, NJ, 2, 4, 32), dtype=np.float32)
            for t in range(2):
                for ii in range(16):
                    # [bp, j, k] -> [k, j, bp]
                    z[ii, :, :, t, :, t * 16 + ii] = (
                        xq[:, :, t, ii, :].transpose(2, 1, 0))
            bds.append(np.ascontiguousarray(
                z.reshape(128, NJ, 2, 128)).astype(f8))
        m = {"ws8": ws8, "wsT1": wsT1, "wsT2": wsT2,
             "ws160": ws16[:, :, 0:NJ // 2, :].copy(),
             "ws161": ws16[:, :, NJ // 2:, :].copy(), "xt16": xt16,
             "id32": id32, "mskf": mskf, "o40x": o40x, "mskS": mskS,
             "dm0": dm0, "dm1": dm1}
        for q in range(NBB):
            m[f"bd{q}"] = bds[q]
        m["bd0a"] = bds[0][:, 0:NJ // 2].copy()
        in_maps.append(m)
    return in_maps


_NC_CACHE = {}


def kernel(x, W):
    from concourse.bass_utils import run_bass_kernel_spmd

    if "nc" not in _NC_CACHE:
        _NC_CACHE["nc"] = build_program()
    nc = _NC_CACHE["nc"]
    in_maps = _host_inputs(x, W)
    res = run_bass_kernel_spmd(nc, in_maps, core_ids=list(range(N_CORES)))
    out = np.concatenate([r["out"] for r in res.results], axis=0)
    return out.reshape(B_FULL, OC, OD).astype(np.float32)


if __name__ == "__main__":
    nc = build_program()
    print("program built ok,",
          sum(len(b.instructions) for f in nc.m.functions for b in f.blocks),
          "instructions")


# revision 5
# speedup vs baseline: 1.2594x; 1.0307x over previous
"""Trainium2 Bass kernel for the CapsuleNet dynamic-routing layer, v2.

Math (per batch element b):
    u_hat[b,i,o,d] = sum_k W[i,o,d,k] * x[b,i,k]   # B=256, IC=1152, OC=10, OD=16, ID=8
    b_log = 0
    for it in 0..2:
        c = softmax(b_log, axis=o); s = sum_i c*u_hat; v = squash(s)
        if it < 2: b_log += sum_d u_hat * v

v2 design (vs the v1 DVE-agreement kernel):
  * Data-parallel over B across 8 cores (32 local rows), W replicated.
  * Partition layout p=(bp4, ii32): b-blocks of 4 (bb in 0..8), i-blocks of 32
    (j in 0..36).  All heavy contractions are fp8e4 DoubleRow matmuls on PE.
  * s is split s = s0 + delta-terms: s0 = 0.1*sum_i u_hat comes from an fp16
    pass-1 (x16 @ W16, full precision); everything delta-scaled (0.4% of s)
    runs in fp8 (error-tolerant).
  * The agreement (b_inc = sum_d u_hat*v) moves from DVE onto the PE via a
    transposed fp8 copy u_hatT[(o,d), (bp,ii)] built directly by ws8 x bd
    matmuls.  Agreement output lands as psum [128,(bp',o')=40+4] per (j,bb)
    whose 40 columns ARE the c-blockdiag positions for stage-1: no DRAM
    bounces at all.  4 extra M-columns compute sum_o b_inc/10 in the same
    matmul, so softmax is LINEARIZED (|logits|~4e-3): delta = (b_inc -
    mean_o)*0.1, computed by one DVE subtract + one mask-multiply.
  * Stage-1 s-increments accumulate across routing iterations in a [32,160]
    psum (delta_t are increments); one squash per round.
  * o in 10 = 8+2: u_hatT keeps (o<8,d) on 128 partitions (plane t0) and
    (o>=8,d) 4-up packed over j%4 (plane t1); the agreement DoubleRow matmul
    sums both planes with per-j variable-stride APs and per-class vd masks.
"""

import sys

sys.path.insert(0, "/opt/trn_rl_repo")

from contextlib import ExitStack

import numpy as np

import bass_rust
import concourse.bass as bass
import concourse.tile as tile
from concourse import mybir

BF = mybir.dt.float16
F8 = mybir.dt.float8e4
F32 = mybir.dt.float32
AX = mybir.AxisListType
AF = mybir.ActivationFunctionType
DR = mybir.MatmulPerfMode.DoubleRow

N_CORES = 8
B_FULL, IC, OC, OD, ID = 256, 1152, 10, 16, 8
B_LOC = B_FULL // N_CORES            # 32
F = OC * OD                          # 160
NJ = 36                              # i-blocks of 32
NJQ = 9                              # jq groups of 4 j-blocks
NBB = 8                              # b-blocks of 4
SW = 64.0                            # W scale into fp8
SV = 32.0                            # v scale into fp8
PSA = 2048.0                         # agreement psum scale = SW*SV
DSC = 1024.0                         # delta fp8 scale


def _squash(nc, smp, ps, scale, vout, P=B_LOC):
    """vout = squash(scale * ps) with ps a [P, 160] psum slab (f32).

    squash(s) = n2/((1+n2)(n+eps)) * s with n2 = |s|^2 per (b,o).
    """
    sq = smp.tile([P, F], F32, tag=f"sq{P}", name="sq")
    nc.scalar.activation(sq[:], ps[:], AF.Square, scale=float(scale))
    n2 = smp.tile([P, OC], F32, tag=f"n2{P}", name="n2")
    nc.vector.tensor_reduce(
        n2[:], sq[:].rearrange("p (o d) -> p o d", d=OD), axis=AX.X,
        op=mybir.AluOpType.add)
    n1 = smp.tile([P, OC], F32, tag=f"n1{P}", name="n1")
    nc.scalar.add(n1[:], n2[:], 1.0)
    sn = smp.tile([P, OC], F32, tag=f"sn{P}", name="sn")
    nc.scalar.sqrt(sn[:], n2[:])
    t1 = smp.tile([P, OC], F32, tag=f"t1{P}", name="t1")
    nc.vector.tensor_mul(t1[:], n1[:], sn[:])
    r1 = smp.tile([P, OC], F32, tag=f"r1{P}", name="r1")
    nc.vector.reciprocal(r1[:], t1[:])
    f1 = smp.tile([P, OC], F32, tag=f"f1{P}", name="f1")
    nc.vector.tensor_mul(f1[:], n2[:], r1[:])
    if scale != 1.0:
        nc.scalar.mul(f1[:], f1[:], float(scale))
    nc.vector.tensor_mul(
        vout[:].rearrange("p (o d) -> p o d", d=OD),
        ps[:].rearrange("p (o d) -> p o d", d=OD),
        f1[:].unsqueeze(-1).broadcast_to((P, OC, OD)))


def _split_multiwait(nc):
    """Walrus encodes at most ONE semaphore wait on Matmult/Ldweights and
    HWDGE DMACopy instructions.  Hoist excess waits onto same-engine NoOps."""
    for fn in nc.m.functions:
        for bb in fn.blocks:
            out = []
            k = 0
            for ins in bb.instructions:
                si = ins.sync_info
                waits = list(si.on_wait) if si is not None and si.on_wait else []
                limit = 1
                if ins.opcode == "DMACopy":
                    q = str(getattr(ins, "queue", "") or "")
                    if "HW" in q and len(waits) > 1:
                        raise AssertionError(
                            f"HWDGE DMA {ins.name} has {len(waits)} waits: {ins}")
                if len(waits) > limit:
                    for w in waits[:-limit]:
                        nop = mybir.InstNoOp(name=f"{ins.name}-wn{k}", ins=[], outs=[])
                        k += 1
                        nop.engine = ins.engine
                        nop.sync_info = mybir.SyncInfo(on_wait=[w], on_update=[])
                        out.append(nop)
                    ins.sync_info = mybir.SyncInfo(
                        on_wait=waits[-limit:],
                        on_update=list(si.on_update) if si.on_update else [])
                out.append(ins)
            bb.instructions = out


class _DrainBalancer:
    """Greedy min-load assignment of psum drains / elementwise ops to
    ACT/DVE/Pool using modeled per-op ns."""

    def __init__(self, nc, abias=1.0):
        self.nc = nc
        self.abias = abias
        self.pbias = 1.0
        # pre-charge engines for fixed work emitted outside the balancer
        # (squash chains + vd builds on DVE, misc on ACT/Pool)
        self.load = {"a": 3000.0, "d": 14000.0, "p": 2000.0}

    def _pick(self, costs):
        e = min(costs, key=lambda k: self.load[k] + costs[k])
        self.load[e] += costs[e]
        return e

    def drain(self, dst, src, nelem):
        # GPSIMD cannot access PSUM on hardware: drains are ACT/DVE only.
        # Bias toward ACT, which cannot run tensor_tensor work.
        costs = {"a": (nelem * 0.833 + 143) * self.abias,
                 "d": nelem * 1.042 + 125}
        e = self._pick(costs)
        if e == "a":
            self.nc.scalar.copy(dst, src)
        else:
            self.nc.vector.tensor_copy(dst, src)

    def tt(self, op, out, in0, in1, nelem, x2=False, psum=False):
        """tensor_tensor op on DVE or Pool (ACT cannot; Pool not on psum)."""
        costs = {"d": nelem * 1.042 * (0.5 if x2 else 1.0) + 125,
                 "p": (nelem * 1.984 + 95) * self.pbias}
        if psum:
            del costs["p"]
        e = self._pick(costs)
        eng = self.nc.vector if e == "d" else self.nc.gpsimd
        getattr(eng, op)(out, in0, in1)


def _ap_pair(sl, off0, stride_t, n):
    """[128, 2, n] AP over a [128, X] slice: t=0 at off0, t=1 at off0+stride_t."""
    ap = sl[:, off0:off0 + n].copy()
    ap.ap = bass_rust.VecI64Pair([list(ap.ap[0]), [stride_t, 2], [1, n]])
    return ap


def build_program(split_waits=True, cfg=None):
    cfg = cfg or {}
    PB = cfg.get('pb', 4)
    PAGA = cfg.get('paga', 1)
    PT2 = cfg.get('pt2', 2)
    PAGB = cfg.get('pagb', 4)
    PPS1 = cfg.get('pps1', 2)
    PRED = cfg.get('pre_d', 0.0)
    ABIAS = cfg.get('abias', 1.15)
    PBA = cfg.get('pb_a', 0.55)
    PBB = cfg.get('pb_b', 1.8)
    nc = bass.Bass()
    # --- DRAM parameters ---
    bd_d = [nc.declare_dram_parameter(f"bd{q}", [128, NJ, 2, 128], F8,
                                      isOutput=False) for q in range(NBB)]
    bd0a_d = nc.declare_dram_parameter("bd0a", [128, NJ // 2, 2, 128], F8,
                                       isOutput=False)
    ws8_d = nc.declare_dram_parameter("ws8", [128, NJ, 2, F], F8, isOutput=False)
    wsT1_d = nc.declare_dram_parameter("wsT1", [128, NJ, 2, 128], F8, isOutput=False)
    wsT2_d = nc.declare_dram_parameter("wsT2", [128, NJ, 2, 32], F8, isOutput=False)
    ws16_d = [nc.declare_dram_parameter(f"ws16{h}", [128, 2, NJ // 2, F], BF,
                                        isOutput=False) for h in range(2)]
    xt16_d = nc.declare_dram_parameter("xt16", [128, 2, NJ, B_LOC], BF, isOutput=False)
    id32_d = nc.declare_dram_parameter("id32", [32, 32], BF, isOutput=False)
    mskf_d = nc.declare_dram_parameter("mskf", [40, F], BF, isOutput=False)
    o40x_d = nc.declare_dram_parameter("o40x", [40, NBB, 32], BF, isOutput=False)
    mskS_d = nc.declare_dram_parameter("mskS", [128, 40, NJ], BF, isOutput=False)
    dm0_d = nc.declare_dram_parameter("dm0", [128, 40], BF, isOutput=False)
    dm1_d = nc.declare_dram_parameter("dm1", [32, 40], BF, isOutput=False)
    out_d = nc.declare_dram_parameter("out", [B_LOC, F], F32, isOutput=True)

    with ExitStack() as ctx:
        tc = ctx.enter_context(tile.TileContext(nc))
        st = ctx.enter_context(tc.tile_pool(name="st", bufs=1))
        bdp = ctx.enter_context(tc.tile_pool(name="bdp", bufs=2))
        tmp = ctx.enter_context(tc.tile_pool(name="tmp", bufs=2))
        smp = ctx.enter_context(tc.tile_pool(name="smp", bufs=4))
        mkd = ctx.enter_context(tc.tile_pool(name="mkd", bufs=2))

        # --- persistent SBUF ---
        ws8 = st.tile([128, NJ, 2, F], F8, tag="ws8", name="ws8")
        wsT1 = st.tile([128, NJ, 2, 128], F8, tag="wsT1", name="wsT1")
        wsT2 = st.tile([128, NJ, 2, 32], F8, tag="wsT2", name="wsT2")
        ws16 = st.tile([128, 2, NJ // 2, F], BF, tag="ws16", name="ws16")
        xt16 = st.tile([128, 2, NJ, B_LOC], BF, tag="xt16", name="xt16")
        u8 = st.tile([128, NJ // 2, NBB, 2, F], F8, tag="u8", name="u8")
        uT = st.tile([128, NJQ, NBB, 512], F8, tag="uT", name="uT")
        uT2 = st.tile([32, NJQ, NBB, 4, 128], F8, tag="uT2", name="uT2")
        id32 = st.tile([32, 32], BF, tag="id32", name="id32")
        mskf = st.tile([40, F], BF, tag="mskf", name="mskf")
        o40x = st.tile([40, NBB, 32], BF, tag="o40x", name="o40x")
        mskS = st.tile([128, 40, NJ], BF, tag="mskS", name="mskS")
        dm0 = st.tile([128, 40], BF, tag="dm0", name="dm0")
        dm1 = st.tile([32, 40], BF, tag="dm1", name="dm1")
        s_sb = [st.tile([B_LOC, F], BF, tag=f"s_sb{r}", name=f"s_sb{r}")
                for r in range(2)]
        v32 = [st.tile([B_LOC, F], BF, tag=f"v32_{r}", name=f"v32_{r}")
               for r in range(2)]
        vT1 = st.tile([128, 32], BF, tag="vT1", name="vT1")
        vT2 = st.tile([128, 32], BF, tag="vT2", name="vT2")
        vd = st.tile([128, NBB, 40], F8, tag="vd", name="vd")
        vd2 = st.tile([32, NBB, 40], F8, tag="vd2", name="vd2")
        of32 = st.tile([B_LOC, F], F32, tag="of32", name="of32")
        delta = [st.tile([128, NJ // 2, 2, 48], F8, tag=f"delta{bb}",
                         name=f"delta{bb}")
                 for bb in range(NBB)]

        # --- input loads (HWDGE, no waits) ---
        nc.sync.dma_start(out=xt16[:], in_=xt16_d[:])
        nc.sync.dma_start(out=ws8[:], in_=ws8_d[:])
        nc.sync.dma_start(out=wsT1[:], in_=wsT1_d[:])
        nc.sync.dma_start(out=wsT2[:], in_=wsT2_d[:])
        nc.scalar.dma_start(out=ws16[:], in_=ws16_d[0][:])
        nc.sync.dma_start(out=id32[:], in_=id32_d[:])
        nc.sync.dma_start(out=mskf[:], in_=mskf_d[:])
        nc.sync.dma_start(out=mskS[:], in_=mskS_d[:])
        nc.sync.dma_start(out=o40x[:], in_=o40x_d[:])
        nc.sync.dma_start(out=dm0[:], in_=dm0_d[:])
        nc.sync.dma_start(out=dm1[:], in_=dm1_d[:])
        nc.scalar.memzero(vd[:])
        for bb in range(NBB):
            nc.scalar.memzero(delta[bb][:, :, :, 40:48])

        bal = _DrainBalancer(nc, abias=ABIAS)
        bal.pbias = PBA
        bal.load['d'] = PRED

        def vd_build(vsrc, pvt_pool, pvt_tag):
            """vd[:, bb, 0:44]=t0 / 44*(1+c):...=t1 planes from v [32,160]."""
            pv1 = pvt_pool.tile([128, 32], BF, tag=pvt_tag, name="pv1")
            nc.tensor.transpose(pv1[:], vsrc[:, 0:128], id32[:])
            nc.scalar.activation(vT1[:], pv1[:], AF.Copy, scale=SV)
            pv2 = pvt_pool.tile([128, 32], BF, tag=pvt_tag, name="pv2")
            nc.tensor.transpose(pv2[0:32, :], vsrc[:, 128:160], id32[:])
            nc.scalar.activation(vT2[0:32, :], pv2[0:32, :], AF.Copy, scale=SV)
            # (diag(o) - 0.1) pattern folded into dm0
            in0 = vT1[:].rearrange("p (bb four) -> p bb four", four=4)
            nc.vector.tensor_mul(
                vd[:].rearrange("p bb (four o) -> p bb four o", o=10),
                in0.unsqueeze(-1).broadcast_to((128, NBB, 4, 10)),
                dm0[:].rearrange("p (four o) -> p four o", o=10)
                .unsqueeze(1).broadcast_to((128, NBB, 4, 10)))
            # o in {8,9} plane on 32 partitions
            i2 = vT2[0:32, :].rearrange("p (bb four) -> p bb four", four=4)
            nc.vector.tensor_mul(
                vd2[:].rearrange("p bb (four o) -> p bb four o", o=10),
                i2.unsqueeze(-1).broadcast_to((32, NBB, 4, 10)),
                dm1[:].rearrange("p (four o) -> p four o", o=10)
                .unsqueeze(1).broadcast_to((32, NBB, 4, 10)))

        def agr_jo(bb, jo, pagr):
            """One third of block bb's agreement + its delta mask."""
            if True:
                pag = pagr.tile([128, 12, 40], F32, tag="pag",
                                name=f"pag{bb}_{jo}")
                for jj in range(12):
                    j = 12 * jo + jj
                    jq, jm = j // 4, j % 4
                    nc.tensor.matmul(pag[:, jj, :],
                                     lhsT=uT[:, jq, bb, 128 * jm:128 * (jm + 1)],
                                     rhs=vd[:, bb, :], start=True, stop=False)
                    nc.tensor.matmul(pag[:, jj, :],
                                     lhsT=uT2[:, jq, bb, jm, :],
                                     rhs=vd2[:, bb, :], start=False, stop=True)
                # psum already holds binc - mean_o; GPSIMD can't read psum,
                # so drain (ACT/DVE) then mask (DVE/Pool) from SBUF
                tch = tmp.tile([128, 12, 40], BF, tag="tch", name=f"tch{bb}_{jo}")
                bal.drain(tch[:], pag[:], 480)
                bal.tt("tensor_mul",
                       delta[bb][:, 6 * jo:6 * (jo + 1), :, 0:40]
                       .rearrange("p jp t c -> p (jp t) c"),
                       tch[:], mskS[:, :, 12 * jo:12 * (jo + 1)]
                       .transpose([0, 2, 1]), 480)

        def agr_round(bb, pagr):
            for jo in range(NJQ // 3):
                agr_jo(bb, jo, pagr)

        def stage1(r, bb, pps1, psv, tag="ps1"):
            ps1 = pps1.tile([48, F], F32, tag=tag, name=f"ps1_{r}_{bb}")
            for jp in range(NJ // 2):
                lt = delta[bb][:, jp, :, :]
                nc.tensor.matmul(ps1[:], lhsT=lt, rhs=u8[:, jp, bb, :, :],
                                 start=(jp == 0), stop=(jp == NJ // 2 - 1),
                                 perf_mode=DR)
            md = mkd.tile([40, F], BF, tag="mkd", name=f"mkd_{r}_{bb}")
            bal.tt("tensor_mul", md[:], ps1[0:40, :], mskf[:], F, psum=True)
            nc.tensor.matmul(psv[:], lhsT=o40x[:, bb, :], rhs=md[:],
                             start=False, stop=(bb == NBB - 1))
            return ps1

        # ================= phase A: builds + iter-0 =================
        with tc.tile_pool(name="pb", bufs=PB, space="PSUM") as pb, \
             tc.tile_pool(name="pT2", bufs=PT2, space="PSUM") as pT2, \
             tc.tile_pool(name="pagrA", bufs=PAGA, space="PSUM") as pagrA, \
             tc.tile_pool(name="ppsvA", bufs=1, space="PSUM") as ppsvA:
            psv1 = ppsvA.tile([B_LOC, F], F32, tag="psv1", name="psv1")
            # pass-1 (fp16): ps0 = SW * sum_i u_hat  -> s0, v0
            # (two j-halves around q0's build so PE fills the load gap;
            #  ps0 borrows the pagrA slot, released before agr0 needs it)
            ps0 = pagrA.tile([B_LOC, F], F32, tag="pag", name="ps0")

            def pass1_half(h):
                for j in range(h * NJ // 2, (h + 1) * NJ // 2):
                    for t in range(2):
                        nc.tensor.matmul(ps0[:], lhsT=xt16[:, t, j, :],
                                         rhs=ws16[:, t, j % (NJ // 2), :],
                                         start=(j == 0 and t == 0),
                                         stop=(j == NJ - 1 and t == 1))
            pass1_half(0)
            pend = []

            def pop_pend(n):
                for _ in range(n):
                    if pend:
                        pend.pop(0)()

            for bb in range(NBB):
                bdt = bdp.tile([128, NJ, 2, 128], F8, tag="bdt", name=f"bdt{bb}")
                # ring-slot reuse gives blocks >=2 multiple waits -> SWDGE
                eng = (nc.scalar, nc.sync, nc.gpsimd, nc.gpsimd, nc.gpsimd,
                       nc.gpsimd, nc.gpsimd, nc.gpsimd)[bb]
                if bb == 0:
                    # split first block's load so early builds start sooner
                    eng.dma_start(out=bdt[:, 0:NJ // 2], in_=bd0a_d[:])
                    eng.dma_start(out=bdt[:, NJ // 2:],
                                  in_=bd_d[0][:, NJ // 2:])
                    nc.gpsimd.dma_start(out=ws16[:], in_=ws16_d[1][:])
                else:
                    eng.dma_start(out=bdt[:], in_=bd_d[bb][:])
                if True:
                    for jq in range(NJQ):
                        p2 = pT2.tile([32, 4, 128], F32, tag="p2",
                                      name=f"p2_{bb}_{jq}")
                        p1 = pb.tile([128, 4, 128], F32, tag="pb", bufs=PB,
                                     name=f"p1{jq}_{bb}")
                        for jh in range(2):
                            jp = 2 * jq + jh
                            pu = pb.tile([128, 2, F], F32, tag="pb", bufs=PB,
                                         name=f"pu{jp}_{bb}")
                            for jm2 in range(2):
                                j = 2 * jp + jm2
                                jm = 2 * jh + jm2
                                bsl = bdt[:, j, :, :]
                                nc.tensor.matmul(pu[:, jm2, :], lhsT=bsl,
                                                 rhs=ws8[:, j, :, :],
                                                 start=True, stop=True,
                                                 perf_mode=DR)
                                nc.tensor.matmul(p1[:, jm, :],
                                                 lhsT=wsT1[:, j, :, :],
                                                 rhs=bsl, start=True,
                                                 stop=True, perf_mode=DR)
                                nc.tensor.matmul(p2[:, jm, :],
                                                 lhsT=wsT2[:, j, :, :],
                                                 rhs=bsl, start=True,
                                                 stop=True, perf_mode=DR)
                            bal.drain(u8[:, jp, bb, :, :], pu[:], 2 * F)
                        bal.drain(uT[:, jq, bb, 0:512], p1[:], 512)
                        bal.drain(uT2[:, jq, bb, :, :], p2[:], 512)
                        if bb > 0:
                            pop_pend(1)
                if bb == 0:
                    pass1_half(1)
                    _squash(nc, smp, ps0, 0.1 / SW, v32[0])
                    nc.scalar.activation(s_sb[0][:], ps0[:], AF.Copy,
                                         scale=0.1 / SW)
                    nc.tensor.matmul(psv1[:], lhsT=id32[:], rhs=s_sb[0][:],
                                     start=True, stop=False)
                    vd_build(v32[0], pagrA, "pag")
                for jo in range(NJQ // 3):
                    pend.append(lambda b=bb, o=jo: agr_jo(b, o, pagrA))
                pend.append(lambda b=bb: stage1(1, b, pagrA, psv1, tag="pag"))
            pop_pend(len(pend))
            # phase boundary: reset cumulative loads so phase-B assignment
            # is phase-local (build-phase totals would otherwise force all
            # masks onto Pool)
            bal.load = {"a": 0.0, "d": 0.0, "p": 0.0}
            bal.pbias = PBB
            _squash(nc, smp, psv1, 1.0, v32[1])
            nc.scalar.copy(s_sb[1][:], psv1[:])

        # ================= phase B: routing rounds =================
        with tc.tile_pool(name="pagrB", bufs=PAGB, space="PSUM") as pagrB, \
             tc.tile_pool(name="pps1", bufs=PPS1, space="PSUM") as pps1, \
             tc.tile_pool(name="ppsv", bufs=1, space="PSUM") as ppsv, \
             tc.tile_pool(name="pvt", bufs=1, space="PSUM") as pvt:
            vd_build(v32[1], pvt, "pvt")
            psv = ppsv.tile([B_LOC, F], F32, tag="psv", name="psv2")
            nc.tensor.matmul(psv[:], lhsT=id32[:], rhs=s_sb[1][:],
                             start=True, stop=False)
            for bb in range(NBB):
                agr_round(bb, pagrB)
                stage1(2, bb, pps1, psv)
            _squash(nc, smp, psv, 1.0, of32)
            nc.gpsimd.dma_start(out=out_d[:], in_=of32[:])

    if split_waits:
        _split_multiwait(nc)
    return nc


def _host_inputs(x, W):
    """Per-core input maps from full x [256,1152,8] f32, W [1,1152,10,16,8]."""
    f8 = mybir.dt.np(F8)
    bf = np.float16
    W0 = np.asarray(W[0], dtype=np.float32) * SW
    # ws[q=(ii16,k8), t, j, (o,d)] = W[j*32+t*16+ii, o, d, k] * SW
    Wr = W0.reshape(NJ, 2, 16, OC, OD, ID)
    ws = np.ascontiguousarray(
        Wr.transpose(2, 5, 1, 0, 3, 4).reshape(128, 2, NJ, F))
    ws16 = ws.astype(bf)
    wsj = np.ascontiguousarray(ws.transpose(0, 2, 1, 3))   # [128, NJ, 2, F]
    ws8 = wsj.astype(f8)
    wsT1 = np.ascontiguousarray(wsj[:, :, :, 0:128]).astype(f8)
    wsT2 = np.ascontiguousarray(wsj[:, :, :, 128:160]).astype(f8)
    id32 = np.eye(32, dtype=bf)
    mskf = np.zeros((40, F), dtype=bf)
    for bp in range(4):
        for o in range(OC):
            mskf[bp * 10 + o, o * OD:(o + 1) * OD] = 1.0
    o40x = np.zeros((40, NBB, 32), dtype=bf)
    for bb in range(NBB):
        for bp in range(4):
            for o in range(OC):
                o40x[bp * 10 + o, bb, bb * 4 + bp] = 1.0 / (SW * DSC)
    mskS = np.zeros((128, 40, NJ), dtype=bf)
    for p in range(128):
        bp = p // 32
        mskS[p, bp * 10:(bp + 1) * 10, :] = 0.1 * DSC / PSA
    dm0 = np.zeros((128, 40), dtype=bf)
    for p in range(128):
        o = p // 16
        for bp in range(4):
            for o2 in range(OC):
                dm0[p, bp * 10 + o2] = (1.0 if o2 == o else 0.0) - 0.1
    dm1 = np.zeros((32, 40), dtype=bf)
    for pp in range(32):
        o = 8 + pp // 16
        for bp in range(4):
            for o2 in range(OC):
                dm1[pp, bp * 10 + o2] = (1.0 if o2 == o else 0.0) - 0.1

    in_maps = []
    for core in range(N_CORES):
        xc = np.asarray(x[core * B_LOC:(core + 1) * B_LOC], dtype=np.float32)
        # xt16[q=(ii,k), t, j, b] = x[b, j*32+t*16+ii, k]
        xr = xc.reshape(B_LOC, NJ, 2, 16, ID)
        xt16 = np.ascontiguousarray(
            xr.transpose(3, 4, 2, 1, 0).reshape(128, 2, NJ, B_LOC)).astype(bf)
        # bd[q][K=(ii16,k8), j, bi, t, m=(bp,ii32)] =
        #   x[(2q+bi)*4+bp, j*32+ii32, k] * (ii32 == t*16+ii16)
        x8 = xc.astype(f8).astype(np.float32)
        bds = []
        for bb in range(NBB):
            xq = x8[4 * bb:4 * (bb + 1)].reshape(4, NJ, 2, 16, ID)
            z = np.zeros((16, ID, NJ, 2, 4, 32), dtype=np.float32)
            for t in range(2):
                for ii in range(16):
                    # [bp, j, k] -> [k, j, bp]
                    z[ii, :, :, t, :, t * 16 + ii] = (
                        xq[:, :, t, ii, :].transpose(2, 1, 0))
            bds.append(np.ascontiguousarray(
                z.reshape(128, NJ, 2, 128)).astype(f8))
        m = {"ws8": ws8, "wsT1": wsT1, "wsT2": wsT2,
             "ws160": ws16[:, :, 0:NJ // 2, :].copy(),
             "ws161": ws16[:, :, NJ // 2:, :].copy(), "xt16": xt16,
             "id32": id32, "mskf": mskf, "o40x": o40x, "mskS": mskS,
             "dm0": dm0, "dm1": dm1}
        for q in range(NBB):
            m[f"bd{q}"] = bds[q]
        m["bd0a"] = bds[0][:, 0:NJ // 2].copy()
        in_maps.append(m)
    return in_maps


_NC_CACHE = {}


def kernel(x, W):
    from concourse.bass_utils import run_bass_kernel_spmd

    if "nc" not in _NC_CACHE:
        _NC_CACHE["nc"] = build_program()
    nc = _NC_CACHE["nc"]
    in_maps = _host_inputs(x, W)
    res = run_bass_kernel_spmd(nc, in_maps, core_ids=list(range(N_CORES)))
    out = np.concatenate([r["out"] for r in res.results], axis=0)
    return out.reshape(B_FULL, OC, OD).astype(np.float32)


if __name__ == "__main__":
    nc = build_program()
    print("program built ok,",
          sum(len(b.instructions) for f in nc.m.functions for b in f.blocks),
          "instructions")
